# revision 1
# baseline (speedup 1.0000x reference)
"""DCNv4 block (cv1 1x1 -> offset/mask proj -> deformable bilinear sampling
-> cv2 1x1 -> BN -> SiLU) as a Bass/Tile kernel for Trainium2.

Strategy
--------
Data-parallel over batch: each of the 8 NeuronCores processes one image.

The deformable sampling is reformulated gather-free: with |off| < 1 the
bilinear sample of kernel point k at (h+kh+off_h, w+kw+off_w) equals
  sum_{i,j in {-1,0,1}} tent(off_h - i) * tent(off_w - j) * V[h+kh+i, w+kw+j]
with tent(t) = max(0, 1-|t|).  Merging all 9 kernel points over absolute
displacements e=(eh,ew) in [-2,2]^2 gives 25 "taps":
  out[p,g,:] = sum_e A_e[p,g] * Vpad[p+e, g, :]
  A_e[p,g]   = sum_k mask_k * tent(off_h - (eh-kh)) * tent(off_w - (ew-kw))
Out-of-image corners are handled exactly by zero-padding Vpad (the reference
drops those corners).

Engine mapping:
 - PE: cv1 / offset-projection / cv2 matmuls (float32r), A^T transposes,
   and the 25-term tap accumulation as identity-weight matmuls accumulating
   into PSUM (f32 accumulation).
 - DVE: tent products, A scatter-build, per-tap elementwise A*V products.
 - ACT: tent relus, PSUM->SBUF copies, BN+SiLU epilogue.
 - GPSIMD: a slice of the tap products, memsets.
 - DMA: a replicating access pattern broadcasts per-group tap maps A_e[g,:]
   (16 partitions) to all 128 partitions (V channels are laid out g-major,
   partition j -> group j//8, so one broadcast serves both channel tiles).

All biases ride the matmuls via an appended ones-row.  BN is folded into
cv2 on the host; the offset projection is folded through cv1 on the host so
offsets are computed from x at full precision independent of V's bf16
rounding.
"""

import sys
import numpy as np

if "/opt/trn_rl_repo" not in sys.path:
    sys.path.insert(0, "/opt/trn_rl_repo")

import ml_dtypes

B, C1, C2, H, W = 8, 256, 256, 64, 64
C = 256
G = 16
Cg = 16
K = 9
HW = H * W           # 4096
PW = W + 4           # 68
PH = H + 4
BN_EPS = 1e-5
TPAD = 32            # taps padded to 32 so (t, g) blocks are 128-aligned

_cache = {}


def _v_perm():
    # vtile vt, partition j  ->  original channel g*16 + c
    perm = []
    for vt in range(2):
        for j in range(128):
            g = j // 8
            c = vt * 8 + (j % 8)
            perm.append(g * Cg + c)
    return np.array(perm, np.int64)


def _om_perm():
    # om channel r (0..431) -> original w_off row
    rows = np.zeros(432, np.int64)
    for r in range(144):
        k, g = r // 16, r % 16
        rows[r] = g * 27 + 2 * k            # off_h (dh)
        rows[144 + r] = g * 27 + 2 * k + 1  # off_w (dw)
        rows[288 + r] = g * 27 + 18 + k     # mask
    return rows


def _split_multiwait(nc, mybir, max_waits=1):
    """walrus in this container rejects >1 sem wait on one instruction;
    split extras onto preceding same-engine NoOps (equivalent ordering)."""
    for f in nc.m.functions:
        for bb in f.blocks:
            out = []
            for inst in bb.instructions:
                si = inst.sync_info
                if si is not None and len(si.on_wait) > max_waits:
                    waits = list(si.on_wait)
                    for w in waits[:-max_waits]:
                        nop = mybir.InstNoOp(
                            name=f"I-nopw{nc.next_id()}", ins=[], outs=[])
                        nop.engine = inst.engine
                        nop.sync_info = mybir.SyncInfo(on_wait=[w], on_update=[])
                        nc.register_instruction(nop)
                        out.append(nop)
                    si.on_wait = waits[-max_waits:]
                out.append(inst)
            bb.instructions = out


def _build_nc(phase=99):
    import concourse.bass as bass
    import concourse.mybir as mybir
    import concourse.tile as tile

    f32 = mybir.dt.float32
    f32r = mybir.dt.float32r
    bf16 = mybir.dt.bfloat16
    ALU = mybir.AluOpType
    ACTF = mybir.ActivationFunctionType

    nc = bass.Bass()

    x_d = nc.dram_tensor("x", [C1, HW], f32r, kind="ExternalInput")
    wt1_d = nc.dram_tensor("wt1", [C1, 256], f32r, kind="ExternalInput")
    wtom_d = nc.dram_tensor("wtom", [C1, 512], f32r, kind="ExternalInput")
    wt2_d = nc.dram_tensor("wt2", [C, C2], bf16, kind="ExternalInput")
    b1_d = nc.dram_tensor("b1", [1, 256], f32r, kind="ExternalInput")
    b2_d = nc.dram_tensor("b2", [C2, 1], f32, kind="ExternalInput")
    bom_d = nc.dram_tensor("bom", [1, 512], f32r, kind="ExternalInput")
    idn_d = nc.dram_tensor("idn", [128, 128], bf16, kind="ExternalInput")
    ones_d = nc.dram_tensor("onesrow", [1, 512], f32r, kind="ExternalInput")
    y_d = nc.dram_tensor("y", [C2, HW], f32, kind="ExternalOutput")

    with tile.TileContext(nc) as tc:
        with tc.tile_pool(name="persist", bufs=1) as persist:

            # ---- persistent tiles ----
            wt1s = [persist.tile([128, 256], f32r, name=f"wt1_{i}") for i in range(2)]
            wtoms = [persist.tile([128, 512], f32r, name=f"wtom_{i}") for i in range(2)]
            wt2s = [persist.tile([128, 256], bf16, name=f"wt2_{i}") for i in range(2)]
            b1row = persist.tile([1, 256], f32r, name="b1row")
            bom1 = persist.tile([1, 512], f32r, name="bom1")
            b2s = [persist.tile([128, 1], f32, name=f"b2_{i}") for i in range(2)]
            ones = persist.tile([1, 512], f32r, name="ones")
            idn = persist.tile([128, 128], bf16, name="idn")
            vpad = [persist.tile([128, PH, PW], bf16, name=f"vpad_{i}") for i in range(2)]
            vpodd = [persist.tile([128, PH * PW], bf16, name=f"vpodd_{i}") for i in range(2)]
            atile = [persist.tile([128, HW], bf16, name=f"atile_{i}") for i in range(4)]
            usb = [persist.tile([128, HW], bf16, name=f"usb_{v}") for v in range(2)]

            for i in range(2):
                nc.sync.dma_start(out=wtoms[i], in_=wtom_d[i * 128:(i + 1) * 128, :])
            nc.sync.dma_start(out=bom1, in_=bom_d[:, :])
            nc.sync.dma_start(out=ones, in_=ones_d[:, :])

            # zero the pad ring of Vpad (interior written by cv1)
            for vt in range(2):
                vp = vpad[vt]
                nc.vector.memset(vp[:, 0:2, :], 0.0)
                nc.vector.memset(vp[:, PH - 2:PH, :], 0.0)
                nc.vector.memset(vp[:, 2:PH - 2, 0:2], 0.0)
                nc.vector.memset(vp[:, 2:PH - 2, PW - 4:PW], 0.0)

            with tc.tile_pool(name="build", bufs=1) as bpool, \
                 tc.tile_pool(name="tbuf", bufs=2) as tbuf, \
                 tc.tile_pool(name="ombuf", bufs=2) as ombuf, \
                 tc.tile_pool(name="atbuf", bufs=2) as atbuf, \
                 tc.tile_pool(name="psB", bufs=2, space="PSUM") as psB, \
                 tc.tile_pool(name="trps", bufs=2, space="PSUM") as trps:

                xs = [bpool.tile([128, HW], f32r, name=f"xs_{i}") for i in range(2)]
                for q4 in range(4):
                    for i in range(2):
                        nc.sync.dma_start(
                            out=xs[i][:, q4 * 1024:(q4 + 1) * 1024],
                            in_=x_d[i * 128:(i + 1) * 128, q4 * 1024:(q4 + 1) * 1024])
                # weights not needed until the first tap quarter load last
                nc.sync.dma_start(out=idn, in_=idn_d[:, :])
                nc.sync.dma_start(out=b1row, in_=b1_d[:, :])
                for i in range(2):
                    nc.sync.dma_start(out=wt1s[i], in_=wt1_d[i * 128:(i + 1) * 128, :])
                    nc.sync.dma_start(out=wt2s[i], in_=wt2_d[i * 128:(i + 1) * 128, :])
                    nc.sync.dma_start(out=b2s[i], in_=b2_d[i * 128:(i + 1) * 128, :])

                # ---- om^T + tents + A-build (chunks of 4 pixel-tiles),
                # interleaved with tap-apply quarters: quarter q consumes the
                # A columns chunks 2q,2q+1 produced, so sampling overlaps the
                # A-map construction ----
                taps = [(eh, ew) for eh in range(-2, 3) for ew in range(-2, 3)]
                if phase < 3:
                    taps = taps[:1]
                QPIX = 1024          # pixels per tap quarter (16 image rows)
                POOL_TAPS = (2, 5, 8, 11, 14, 17, 20, 23)
                cpt = 4
                n_chunk = 8 if phase >= 2 else 0

                def emit_vblock(nt):
                    # cv1 rows nt*8..nt*8+8 for both channel tiles (+bias)
                    for mt in range(2):
                        ps = psB.tile([128, 512], f32, name="omm")
                        for kt in range(2):
                            nc.tensor.matmul(
                                ps, lhsT=wt1s[kt][:, mt * 128:(mt + 1) * 128],
                                rhs=xs[kt][:, nt * 512:(nt + 1) * 512],
                                start=(kt == 0), stop=False)
                        nc.tensor.matmul(
                            ps, lhsT=b1row[0:1, mt * 128:(mt + 1) * 128],
                            rhs=ones[0:1, :], start=False, stop=True)
                        r0v = nt * 8
                        nc.scalar.activation(
                            out=vpad[mt][:, 2 + r0v:2 + r0v + 8, 2:2 + W],
                            in_=ps[:].rearrange("p (r c) -> p r c", c=W),
                            func=ACTF.Copy)

                def emit_vpodd_seg(qq):
                    # vpodd[p, i] = vpad[p, i+1] over padded rows
                    # [qq*16, qq*16+20): covers quarter qq's windows with
                    # 4B-aligned reads for odd-ew taps
                    lo = qq * 16 * PW
                    hi = min((qq * 16 + 20) * PW, PH * PW) - 1
                    for vt in range(2):
                        vflat = vpad[vt][:].rearrange("p a b -> p (a b)")
                        nc.scalar.activation(out=vpodd[vt][:, lo:hi],
                                             in_=vflat[:, lo + 1:hi + 1],
                                             func=ACTF.Copy)

                vblock_sched = {0: [0, 1, 2], 1: [3, 4], 2: [5, 6], 3: [7]}

                with tc.tile_pool(name="abcp", bufs=6) as abcp, \
                     tc.tile_pool(name="prodp", bufs=10) as prodp:

                    for chk in range(n_chunk):
                        om_t = ombuf.tile([128, cpt, 512], f32, name="om_t")
                        for pi in range(cpt):
                            pt = chk * cpt + pi
                            ps = psB.tile([128, 512], f32, name="omm")
                            for kt in range(2):
                                nc.tensor.matmul(
                                    ps, lhsT=xs[kt][:, pt * 128:(pt + 1) * 128],
                                    rhs=wtoms[kt][:, :],
                                    start=(kt == 0), stop=False)
                            nc.tensor.matmul(
                                ps, lhsT=ones[0:1, 0:128],
                                rhs=bom1[0:1, :], start=False, stop=True)
                            nc.scalar.activation(out=om_t[:, pi, :], in_=ps,
                                                 func=ACTF.Copy)

                        oh = om_t[:, :, 0:144]
                        ow = om_t[:, :, 144:288]
                        msk = om_t[:, :, 288:432]

                        th = [tbuf.tile([128, cpt, 144], bf16, name=f"th_{i}") for i in range(3)]
                        tw = [tbuf.tile([128, cpt, 144], bf16, name=f"tw_{i}") for i in range(3)]
                        mbf = tbuf.tile([128, cpt, 144], bf16, name="mbf")

                        # tents (bf16): index 0,1,2 <-> i=-1,0,+1
                        # t(-1)=relu(-o); t(+1)=relu(o); slot1 holds NEGATED
                        # t(0): |o|-1 = relu(o)+relu(-o)-1.  Sign is fixed at
                        # scatter time: terms with exactly one i/j==1 subtract.
                        nc.scalar.activation(out=th[2], in_=oh, func=ACTF.Relu)
                        nc.scalar.activation(out=tw[2], in_=ow, func=ACTF.Relu)
                        nc.scalar.activation(out=th[0], in_=oh, func=ACTF.Relu, scale=-1.0)
                        nc.scalar.activation(out=tw[0], in_=ow, func=ACTF.Relu, scale=-1.0)
                        nc.vector.scalar_tensor_tensor(out=th[1], in0=th[2], scalar=-1.0,
                                                       in1=th[0], op0=ALU.add, op1=ALU.add)
                        nc.vector.scalar_tensor_tensor(out=tw[1], in0=tw[2], scalar=-1.0,
                                                       in1=tw[0], op0=ALU.add, op1=ALU.add)
                        nc.scalar.activation(out=mbf, in_=msk, func=ACTF.Copy)
                        for i in range(3):
                            nc.vector.tensor_tensor(out=th[i], in0=th[i], in1=mbf,
                                                    op=ALU.mult)

                        # A^T chunk [128, cpt, (TPAD t, 16 g)]
                        at = atbuf.tile([128, cpt, TPAD * 16], bf16, name="at")
                        nc.gpsimd.memset(at, 0.0)
                        prod = tbuf.tile([128, cpt, 144], bf16, name="prodb")
                        for i in range(3):
                            for j in range(3):
                                peng = nc.gpsimd if (i, j) in ((0, 0), (2, 2)) else nc.vector
                                peng.tensor_tensor(out=prod, in0=th[i], in1=tw[j],
                                                   op=ALU.mult)
                                a_ap = at[:, :, :]
                                o_ap = bass.AP(
                                    a_ap.tensor,
                                    a_ap.offset + (i * 5 + j) * 16,
                                    [[cpt * TPAD * 16, 128], [TPAD * 16, cpt],
                                     [5 * 16, 3], [1, 48]])
                                p_ap = prod[:, :, :]
                                i_ap = bass.AP(
                                    p_ap.tensor, p_ap.offset,
                                    [[cpt * 144, 128], [144, cpt], [48, 3], [1, 48]])
                                sop = ALU.subtract if (i == 1) != (j == 1) else ALU.add
                                nc.vector.tensor_tensor(out=o_ap, in0=o_ap, in1=i_ap,
                                                        op=sop)

                        # transpose A^T -> A tiles [(t8, g16), pix]
                        for tb in range(4):
                            tps = trps.tile([128, 512], bf16, name="tr")
                            for s in range(4):
                                nc.tensor.transpose(
                                    tps[:, s * 128:(s + 1) * 128],
                                    at[:, s, tb * 128:(tb + 1) * 128],
                                    idn[:, :])
                            col = chk * cpt * 128
                            nc.scalar.activation(
                                out=atile[tb][:, col:col + 512], in_=tps,
                                func=ACTF.Copy)

                        # ---- tap-apply quarter after every odd chunk ----
                        if chk % 2 == 1:
                            for ntv in vblock_sched[chk // 2]:
                                emit_vblock(ntv)
                            emit_vpodd_seg(chk // 2)
                        if phase >= 3 and chk % 2 == 1:
                            qq = chk // 2
                            r0 = qq * 16
                            with tc.tile_pool(name=f"ups{qq}", bufs=1,
                                              space="PSUM") as upsp:
                                ups = [upsp.tile([128, QPIX], f32,
                                                 name=f"ups_{qq}_{v}")
                                       for v in range(2)]
                                for t, (eh, ew) in enumerate(taps):
                                    tb, ts = t // 8, t % 8
                                    abc = abcp.tile([128, QPIX], bf16, name="abc")
                                    a_ap = atile[tb][:, :]
                                    sap = bass.AP(
                                        a_ap.tensor,
                                        a_ap.offset + ts * 16 * HW + qq * QPIX,
                                        [[HW, 16], [0, 8], [1, QPIX]])
                                    nc.sync.dma_start(out=abc, in_=sap)
                                    abc3 = abc[:].rearrange("p (h w) -> p h w", w=W)
                                    for vt in range(2):
                                        if ew % 2 == 0:
                                            win = vpad[vt][:,
                                                           2 + r0 + eh:2 + r0 + eh + 16,
                                                           2 + ew:2 + ew + W]
                                        else:
                                            vp3 = vpodd[vt][:].rearrange(
                                                "p (a b) -> p a b", b=PW)
                                            win = vp3[:,
                                                      2 + r0 + eh:2 + r0 + eh + 16,
                                                      1 + ew:1 + ew + W]
                                        eng = (nc.gpsimd
                                               if (t in POOL_TAPS and vt == 1)
                                               or (t in (6, 18) and vt == 0)
                                               else nc.vector)
                                        pr = prodp.tile([128, QPIX], bf16, name="tp")
                                        eng.tensor_tensor(
                                            out=pr[:].rearrange("p (h w) -> p h w", w=W),
                                            in0=abc3, in1=win, op=ALU.mult)
                                        for nb in range(2):
                                            nc.tensor.matmul(
                                                ups[vt][:, nb * 512:(nb + 1) * 512],
                                                lhsT=idn[:, :],
                                                rhs=pr[:, nb * 512:(nb + 1) * 512],
                                                start=(t == 0),
                                                stop=(t == len(taps) - 1))
                                for vt in range(2):
                                    nc.scalar.activation(
                                        out=usb[vt][:, qq * QPIX:(qq + 1) * QPIX],
                                        in_=ups[vt], func=ACTF.Copy)

                            # cv2 + BN + SiLU for this quarter's pixel columns
                            if phase >= 4:
                                with tc.tile_pool(name=f"cvps{qq}", bufs=2,
                                                  space="PSUM") as cvps, \
                                     tc.tile_pool(name=f"ysb{qq}", bufs=2) as ysbp:
                                    for nt in (2 * qq, 2 * qq + 1):
                                        for mt in range(2):
                                            ps2 = cvps.tile([128, 512], f32,
                                                            name="cv2ps")
                                            for kt in range(2):
                                                nc.tensor.matmul(
                                                    ps2,
                                                    lhsT=wt2s[kt][:, mt * 128:(mt + 1) * 128],
                                                    rhs=usb[kt][:, nt * 512:(nt + 1) * 512],
                                                    start=(kt == 0), stop=(kt == 1))
                                            ysb = ysbp.tile([128, 512], f32,
                                                            name="ysb")
                                            nc.scalar.activation(
                                                out=ysb, in_=ps2, func=ACTF.Silu,
                                                bias=b2s[mt][:, 0:1], scale=1.0)
                                            nc.sync.dma_start(
                                                out=y_d[mt * 128:(mt + 1) * 128,
                                                        nt * 512:(nt + 1) * 512],
                                                in_=ysb)

    _split_multiwait(nc, mybir)
    return nc


def _prepare(inputs):
    x = np.ascontiguousarray(np.asarray(inputs["x"], np.float32))
    w_cv1 = np.asarray(inputs["w_cv1"], np.float32)
    b_cv1 = np.asarray(inputs["b_cv1"], np.float32)
    w_off = np.asarray(inputs["w_off"], np.float32)
    b_off = np.asarray(inputs["b_off"], np.float32)
    w_cv2 = np.asarray(inputs["w_cv2"], np.float32)
    bn_g = np.asarray(inputs["bn_gamma"], np.float32)
    bn_b = np.asarray(inputs["bn_beta"], np.float32)
    bn_m = np.asarray(inputs["bn_mean"], np.float32)
    bn_v = np.asarray(inputs["bn_var"], np.float32)

    perm_v = _v_perm()
    W1p = w_cv1[perm_v, :]
    b1p = b_cv1[perm_v]

    Wom = w_off @ w_cv1
    bom = w_off @ b_cv1 + b_off
    omp = _om_perm()
    Wom_big = np.zeros((512, C1), np.float32)
    Wom_big[:432] = Wom[omp]
    bom_big = np.zeros((512,), np.float32)
    bom_big[:432] = bom[omp]

    s = bn_g / np.sqrt(bn_v + BN_EPS)
    W2s = w_cv2 * s[:, None]
    b2f = bn_b - bn_m * s
    W2p = W2s[:, perm_v]

    shared = dict(
        wt1=np.ascontiguousarray(W1p.T),
        wtom=np.ascontiguousarray(Wom_big.T),
        wt2=np.ascontiguousarray(W2p.T).astype(ml_dtypes.bfloat16),
        b1=np.ascontiguousarray(b1p[None, :]),
        b2=np.ascontiguousarray(b2f[:, None]),
        bom=np.ascontiguousarray(bom_big[None, :]),
        idn=np.eye(128, dtype=ml_dtypes.bfloat16),
        onesrow=np.ones((1, 512), np.float32),
    )
    in_maps = []
    for b in range(B):
        m = dict(shared)
        m["x"] = np.ascontiguousarray(x[b].reshape(C1, HW))
        in_maps.append(m)
    return in_maps


def kernel(**inputs):
    from concourse.bass_utils import run_bass_kernel_spmd

    if "nc" not in _cache:
        _cache["nc"] = _build_nc()
    nc = _cache["nc"]
    in_maps = _prepare(inputs)
    res = run_bass_kernel_spmd(nc, in_maps, core_ids=list(range(B)))
    out = np.stack([r["y"].reshape(C2, H, W) for r in res.results])
    return out.astype(np.float32)


if __name__ == "__main__":
    rng = np.random.default_rng(0)
    demo = dict(
        x=rng.standard_normal((B, C1, H, W)).astype(np.float32),
        w_cv1=rng.standard_normal((C, C1)).astype(np.float32) / 16,
        b_cv1=(rng.standard_normal((C,)) * 0.1).astype(np.float32),
        w_off=(rng.standard_normal((G * 3 * K, C)) * 0.01).astype(np.float32),
        b_off=(rng.standard_normal((G * 3 * K,)) * 0.01).astype(np.float32),
        w_cv2=rng.standard_normal((C2, C)).astype(np.float32) / 16,
        bn_gamma=rng.uniform(0.5, 1.5, (C2,)).astype(np.float32),
        bn_beta=(rng.standard_normal((C2,)) * 0.1).astype(np.float32),
        bn_mean=(rng.standard_normal((C2,)) * 0.1).astype(np.float32),
        bn_var=rng.uniform(0.5, 1.5, (C2,)).astype(np.float32),
    )
    y = kernel(**demo)
    print("kernel ran, output", y.shape, y.dtype)



# revision 8
# speedup vs baseline: 1.0601x; 1.0601x over previous
"""DCNv4 block (cv1 1x1 -> offset/mask proj -> deformable bilinear sampling
-> cv2 1x1 -> BN -> SiLU) as a Bass/Tile kernel for Trainium2.

Strategy
--------
Data-parallel over batch: each of the 8 NeuronCores processes one image.

The deformable sampling is reformulated gather-free: with |off| < 1 the
bilinear sample of kernel point k at (h+kh+off_h, w+kw+off_w) equals
  sum_{i,j in {-1,0,1}} tent(off_h - i) * tent(off_w - j) * V[h+kh+i, w+kw+j]
with tent(t) = max(0, 1-|t|).  Merging all 9 kernel points over absolute
displacements e=(eh,ew) in [-2,2]^2 gives 25 "taps":
  out[p,g,:] = sum_e A_e[p,g] * Vpad[p+e, g, :]
  A_e[p,g]   = sum_k mask_k * tent(off_h - (eh-kh)) * tent(off_w - (ew-kw))
Out-of-image corners are handled exactly by zero-padding Vpad (the reference
drops those corners).

Engine mapping:
 - PE: cv1 / offset-projection / cv2 matmuls (float32r), A^T transposes,
   and the 25-term tap accumulation as identity-weight matmuls accumulating
   into PSUM (f32 accumulation).
 - DVE: tent products, A scatter-build, most per-tap elementwise A*V
   products (both 128-channel tiles fused into one [128,2048] op via a
   stride-0 repeat on the A operand).
 - ACT: tents read the offset projection directly from PSUM; cv1 bias is
   applied by the PSUM->SBUF copy's per-partition bias; BN+SiLU epilogue.
 - GPSIMD: 7 of 25 tap products per quarter, 2 A-build products, memsets
   (trimmed to only the A slots not freshly written by the (0,0) scatter,
   which is a copy instead of an accumulate).
 - DMA: a replicating access pattern broadcasts per-group tap maps A_e[g,:]
   (16 partitions) to all 128 partitions (V channels are laid out g-major,
   partition j -> group j//8, so one broadcast serves both channel tiles).

BN is folded into cv2 on the host; the offset projection is folded through
cv1 on the host so offsets are computed from x at full precision independent
of V's bf16 rounding.  The output DMA is bf16 (cast to f32 on host).
"""

import sys
import numpy as np

if "/opt/trn_rl_repo" not in sys.path:
    sys.path.insert(0, "/opt/trn_rl_repo")

import ml_dtypes

B, C1, C2, H, W = 8, 256, 256, 64, 64
C = 256
G = 16
Cg = 16
K = 9
HW = H * W           # 4096
PW = W + 4           # 68
PH = H + 4
BN_EPS = 1e-5
TPAD = 32            # taps padded to 32 so (t, g) blocks are 128-aligned
OM = 432             # offset/mask projection width (3*K*G)

_cache = {}


def _v_perm():
    # vtile vt, partition j  ->  original channel g*16 + c
    perm = []
    for vt in range(2):
        for j in range(128):
            g = j // 8
            c = vt * 8 + (j % 8)
            perm.append(g * Cg + c)
    return np.array(perm, np.int64)


def _om_perm():
    # om channel r (0..431) -> original w_off row
    rows = np.zeros(432, np.int64)
    for r in range(144):
        k, g = r // 16, r % 16
        rows[r] = g * 27 + 2 * k            # off_h (dh)
        rows[144 + r] = g * 27 + 2 * k + 1  # off_w (dw)
        rows[288 + r] = g * 27 + 18 + k     # mask
    return rows


def _split_multiwait(nc, mybir, max_waits=1):
    """walrus in this container rejects >1 sem wait on one instruction;
    split extras onto preceding same-engine NoOps (equivalent ordering)."""
    for f in nc.m.functions:
        for bb in f.blocks:
            out = []
            for inst in bb.instructions:
                si = inst.sync_info
                if si is not None and len(si.on_wait) > max_waits:
                    waits = list(si.on_wait)
                    for w in waits[:-max_waits]:
                        nop = mybir.InstNoOp(
                            name=f"I-nopw{nc.next_id()}", ins=[], outs=[])
                        nop.engine = inst.engine
                        nop.sync_info = mybir.SyncInfo(on_wait=[w], on_update=[])
                        nc.register_instruction(nop)
                        out.append(nop)
                    si.on_wait = waits[-max_waits:]
                out.append(inst)
            bb.instructions = out


def _build_nc(phase=99, pool_taps_q=((2, 5, 9, 12, 15, 19, 22),) * 3 + ((2, 9, 15, 19, 22),),
              startup_pool_prods=((1, 1), (2, 2), (0, 1), (1, 0))):
    import concourse.bass as bass
    import concourse.mybir as mybir
    import concourse.tile as tile

    f32 = mybir.dt.float32
    f32r = mybir.dt.float32r
    bf16 = mybir.dt.bfloat16
    ALU = mybir.AluOpType
    ACTF = mybir.ActivationFunctionType

    nc = bass.Bass()

    x_d = nc.dram_tensor("x", [C1, HW], f32r, kind="ExternalInput")
    wt1_d = nc.dram_tensor("wt1", [C1, 256], f32r, kind="ExternalInput")
    wtom_d = nc.dram_tensor("wtom", [C1, OM], f32r, kind="ExternalInput")
    wt2_d = nc.dram_tensor("wt2", [C, C2], bf16, kind="ExternalInput")
    b1_d = nc.dram_tensor("b1", [C, 1], f32, kind="ExternalInput")
    b2_d = nc.dram_tensor("b2", [C2, 1], f32, kind="ExternalInput")
    bom_d = nc.dram_tensor("bom", [1, OM], f32r, kind="ExternalInput")
    idn_d = nc.dram_tensor("idn", [128, 128], bf16, kind="ExternalInput")
    ones_d = nc.dram_tensor("onesrow", [1, 128], f32r, kind="ExternalInput")
    y_d = nc.dram_tensor("y", [C2, HW], bf16, kind="ExternalOutput")

    with tile.TileContext(nc) as tc:
        with tc.tile_pool(name="persist", bufs=1) as persist:

            # ---- persistent tiles ----
            wt1s = [persist.tile([128, 256], f32r, name=f"wt1_{i}") for i in range(2)]
            wtoms = [persist.tile([128, OM], f32r, name=f"wtom_{i}") for i in range(2)]
            wt2s = [persist.tile([128, 256], bf16, name=f"wt2_{i}") for i in range(2)]
            b1s = [persist.tile([128, 1], f32, name=f"b1_{i}") for i in range(2)]
            bom1 = persist.tile([1, OM], f32r, name="bom1")
            b2s = [persist.tile([128, 1], f32, name=f"b2_{i}") for i in range(2)]
            ones = persist.tile([1, 128], f32r, name="ones")
            idn = persist.tile([128, 128], bf16, name="idn")
            # V and its 1-col-shifted copy, both channel tiles fused:
            # [128 part, vt, padded rows, padded cols]
            vpad = persist.tile([128, 2, PH, PW], bf16, name="vpad")
            vpodd = persist.tile([128, 2, PH * PW], bf16, name="vpodd")
            atile = [persist.tile([128, HW], bf16, name=f"atile_{i}") for i in range(4)]

            for i in range(2):
                nc.sync.dma_start(out=wtoms[i], in_=wtom_d[i * 128:(i + 1) * 128, :])
            nc.sync.dma_start(out=bom1, in_=bom_d[:, :])
            nc.sync.dma_start(out=ones, in_=ones_d[:, :])

            # zero the pad ring of Vpad (interior written by cv1)
            nc.vector.memset(vpad[:, :, 0:2, :], 0.0)
            nc.vector.memset(vpad[:, :, PH - 2:PH, :], 0.0)
            nc.vector.memset(vpad[:, :, 2:PH - 2, 0:2], 0.0)
            nc.vector.memset(vpad[:, :, 2:PH - 2, PW - 4:PW], 0.0)

            with tc.tile_pool(name="build", bufs=1) as bpool, \
                 tc.tile_pool(name="tbuf", bufs=2) as tbuf, \
                 tc.tile_pool(name="ombuf", bufs=2) as ombuf, \
                 tc.tile_pool(name="prodab", bufs=4) as prodab, \
                 tc.tile_pool(name="atbuf", bufs=2) as atbuf, \
                 tc.tile_pool(name="psB", bufs=2, space="PSUM") as psB, \
                 tc.tile_pool(name="trps", bufs=2, space="PSUM") as trps:

                xs = [bpool.tile([128, HW], f32r, name=f"xs_{i}") for i in range(2)]

                def load_xs(q4):
                    for i in range(2):
                        nc.sync.dma_start(
                            out=xs[i][:, q4 * 1024:(q4 + 1) * 1024],
                            in_=x_d[i * 128:(i + 1) * 128, q4 * 1024:(q4 + 1) * 1024])
                load_xs(0)
                nc.sync.dma_start(out=idn, in_=idn_d[:, :])
                for i in range(2):
                    nc.sync.dma_start(out=wt1s[i], in_=wt1_d[i * 128:(i + 1) * 128, :])
                    nc.sync.dma_start(out=b1s[i], in_=b1_d[i * 128:(i + 1) * 128, :])
                load_xs(1)
                for i in range(2):
                    nc.sync.dma_start(out=wt2s[i], in_=wt2_d[i * 128:(i + 1) * 128, :])
                    nc.sync.dma_start(out=b2s[i], in_=b2_d[i * 128:(i + 1) * 128, :])
                load_xs(2)
                load_xs(3)

                taps = [(eh, ew) for eh in range(-2, 3) for ew in range(-2, 3)]
                QPIX = 1024          # pixels per tap quarter (16 image rows)
                cpt = 4

                def emit_vblock(nt):
                    # cv1 rows nt*8..nt*8+8 for both channel tiles; bias is
                    # applied by the PSUM->SBUF activation copy
                    for mt in range(2):
                        ps = psB.tile([128, 512], f32, name="omm")
                        for kt in range(2):
                            nc.tensor.matmul(
                                ps, lhsT=wt1s[kt][:, mt * 128:(mt + 1) * 128],
                                rhs=xs[kt][:, nt * 512:(nt + 1) * 512],
                                start=(kt == 0), stop=(kt == 1))
                        r0v = nt * 8
                        nc.scalar.activation(
                            out=vpad[:, mt, 2 + r0v:2 + r0v + 8, 2:2 + W],
                            in_=ps[:].rearrange("p (r c) -> p r c", c=W),
                            func=ACTF.Identity, bias=b1s[mt][:, 0:1], scale=1.0)

                def emit_vpodd_rows(lo_row, hi_row):
                    # vpodd[p, v, i] = vpad[p, v, i+1]; segment rows chosen so
                    # consecutive segments do not overlap (no WAR with the
                    # previous quarter's odd-tap reads)
                    lo = lo_row * PW
                    hi = min(hi_row * PW, PH * PW) - 1
                    vflat = vpad[:].rearrange("p v a b -> p v (a b)")
                    nc.scalar.activation(out=vpodd[:, :, lo:hi],
                                         in_=vflat[:, :, lo + 1:hi + 1],
                                         func=ACTF.Copy)

                def emit_vpodd_seg(qq):
                    emit_vpodd_rows(qq * 16 + (4 if qq else 0), qq * 16 + 20)

                vblock_sched = {0: [0, 1, 2], 1: [3, 4], 2: [5, 6], 3: [7]}

                def build_chunk_units(chk, pool_prods=((1, 1), (2, 2))):
                    """Closure units for A-map chunk chk (4 pixel-tiles)."""
                    st = {}
                    units = []

                    def u_om(pi):
                        if pi == 0:
                            st["om_t"] = ombuf.tile([128, cpt, OM], bf16,
                                                    name="om_t")
                        pt = chk * cpt + pi
                        ps = psB.tile([128, OM], f32, name="omm")
                        for kt in range(2):
                            nc.tensor.matmul(
                                ps, lhsT=xs[kt][:, pt * 128:(pt + 1) * 128],
                                rhs=wtoms[kt][:, :],
                                start=(kt == 0), stop=False)
                        nc.tensor.matmul(
                            ps, lhsT=ones[0:1, :],
                            rhs=bom1[0:1, :], start=False, stop=True)
                        nc.scalar.activation(out=st["om_t"][:, pi, :], in_=ps,
                                             func=ACTF.Copy)
                    for pi in range(cpt):
                        units.append(lambda pi=pi: u_om(pi))

                    def u_tents():
                        om_t = st["om_t"]
                        oh = om_t[:, :, 0:144]
                        ow = om_t[:, :, 144:288]
                        st["mbf"] = om_t[:, :, 288:432]
                        th = [tbuf.tile([128, cpt, 144], bf16, name=f"th_{i}")
                              for i in range(3)]
                        tw = [tbuf.tile([128, cpt, 144], bf16, name=f"tw_{i}")
                              for i in range(3)]
                        st["th"], st["tw"] = th, tw
                        # tents (bf16): index 0,1,2 <-> i=-1,0,+1
                        # t(-1)=relu(-o); t(+1)=relu(o); slot1 holds NEGATED
                        # t(0): |o|-1 = relu(o)+relu(-o)-1.
                        nc.scalar.activation(out=th[2], in_=oh, func=ACTF.Relu)
                        nc.scalar.activation(out=th[0], in_=oh, func=ACTF.Relu,
                                             scale=-1.0)
                        nc.scalar.activation(out=tw[2], in_=ow, func=ACTF.Relu)
                        nc.scalar.activation(out=tw[0], in_=ow, func=ACTF.Relu,
                                             scale=-1.0)
                        nc.vector.scalar_tensor_tensor(
                            out=th[1], in0=th[2], scalar=-1.0, in1=th[0],
                            op0=ALU.add, op1=ALU.add)
                        nc.vector.scalar_tensor_tensor(
                            out=tw[1], in0=tw[2], scalar=-1.0, in1=tw[0],
                            op0=ALU.add, op1=ALU.add)
                    units.append(u_tents)

                    def u_masks():
                        for i in range(3):
                            nc.vector.tensor_tensor(out=st["th"][i],
                                                    in0=st["th"][i],
                                                    in1=st["mbf"], op=ALU.mult)
                    units.append(u_masks)

                    def u_memset():
                        # (0,0) scatter is a fresh write covering taps r<3,s<3;
                        # zero only the complement
                        at = atbuf.tile([128, cpt, TPAD * 16], bf16, name="at")
                        st["at"] = at
                        a_ap = at[:, :, :]
                        nc.gpsimd.memset(bass.AP(
                            a_ap.tensor, a_ap.offset + 3 * 16,
                            [[cpt * TPAD * 16, 128], [TPAD * 16, cpt],
                             [5 * 16, 3], [1, 32]]), 0.0)
                        nc.gpsimd.memset(bass.AP(
                            a_ap.tensor, a_ap.offset + 15 * 16,
                            [[cpt * TPAD * 16, 128], [TPAD * 16, cpt],
                             [1, 160]]), 0.0)
                    units.append(u_memset)

                    def u_prod(i, j):
                        peng = nc.gpsimd if (i, j) in pool_prods else nc.vector
                        prod = prodab.tile([128, cpt, 144], bf16, name="prodb")
                        peng.tensor_tensor(out=prod, in0=st["th"][i],
                                           in1=st["tw"][j], op=ALU.mult)
                        a_ap = st["at"][:, :, :]
                        o_ap = bass.AP(
                            a_ap.tensor, a_ap.offset + (i * 5 + j) * 16,
                            [[cpt * TPAD * 16, 128], [TPAD * 16, cpt],
                             [5 * 16, 3], [1, 48]])
                        p_ap = prod[:, :, :]
                        i_ap = bass.AP(
                            p_ap.tensor, p_ap.offset,
                            [[cpt * 144, 128], [144, cpt], [48, 3], [1, 48]])
                        if (i, j) == (0, 0):
                            nc.vector.tensor_copy(out=o_ap, in_=i_ap)
                        else:
                            sop = ALU.subtract if (i == 1) != (j == 1) else ALU.add
                            nc.vector.tensor_tensor(out=o_ap, in0=o_ap,
                                                    in1=i_ap, op=sop)
                    for i in range(3):
                        for j in range(3):
                            units.append(lambda i=i, j=j: u_prod(i, j))

                    def u_transpose(tb):
                        tps = trps.tile([128, 512], bf16, name="tr")
                        for s in range(4):
                            nc.tensor.transpose(
                                tps[:, s * 128:(s + 1) * 128],
                                st["at"][:, s, tb * 128:(tb + 1) * 128],
                                idn[:, :])
                        col = chk * cpt * 128
                        nc.scalar.activation(
                            out=atile[tb][:, col:col + 512], in_=tps,
                            func=ACTF.Copy)
                    for tb in range(4):
                        units.append(lambda tb=tb: u_transpose(tb))
                    return units

                def tap_half_units(qq, upsp, st, h=None):
                    """Tap units for quarter qq; h=None -> full 1024-pixel
                    quarter, h=0/1 -> 512-pixel half (own abc DMAs and
                    per-column-region PSUM chains)."""
                    pool_taps = pool_taps_q[qq]
                    units = []
                    r0 = qq * 16

                    def u_tap(t, eh, ew, h):
                        if t == 0 and (h is None or h == 0):
                            st["ups"] = [upsp.tile([128, QPIX], f32,
                                                   name=f"ups_{v}")
                                         for v in range(2)]
                        tb, ts = t // 8, t % 8
                        npix = QPIX if h is None else 512
                        coff = 0 if h is None else h * 512
                        rows = npix // W
                        abc = abcp.tile([128, npix], bf16, name="abc")
                        a_ap2 = atile[tb][:, :]
                        sap = bass.AP(
                            a_ap2.tensor,
                            a_ap2.offset + ts * 16 * HW + qq * QPIX + coff,
                            [[HW, 16], [0, 8], [1, npix]])
                        nc.sync.dma_start(out=abc, in_=sap)
                        # A operand repeated over both channel tiles via a
                        # stride-0 dim
                        b_ap = abc[:, :]
                        abc4 = bass.AP(
                            b_ap.tensor, b_ap.offset,
                            [[npix, 128], [0, 2], [W, rows], [1, W]])
                        rbase = 2 + r0 + (0 if h is None else h * 8) + eh
                        if ew % 2 == 0:
                            win = vpad[:, :, rbase:rbase + rows,
                                       2 + ew:2 + ew + W]
                        else:
                            vp4 = vpodd[:].rearrange("p v (a b) -> p v a b",
                                                     b=PW)
                            win = vp4[:, :, rbase:rbase + rows,
                                      1 + ew:1 + ew + W]
                        eng = nc.gpsimd if t in pool_taps else nc.vector
                        pr = prodp.tile([128, 2, rows, W], bf16, name="tp")
                        eng.tensor_tensor(out=pr, in0=win, in1=abc4,
                                          op=ALU.mult)
                        prf = pr[:].rearrange("p v a b -> p (v a b)")
                        for vt in range(2):
                            for nb in range(npix // 512):
                                nc.tensor.matmul(
                                    st["ups"][vt][:, coff + nb * 512:
                                                  coff + (nb + 1) * 512],
                                    lhsT=idn[:, :],
                                    rhs=prf[:, vt * npix + nb * 512:
                                            vt * npix + (nb + 1) * 512],
                                    start=(t == 0),
                                    stop=(t == len(taps) - 1))
                    for t, (eh, ew) in enumerate(taps):
                        units.append(
                            lambda t=t, e=taps[t], h=h: u_tap(t, e[0], e[1], h))
                    return units

                def u_usb(st, usb_out, h=None):
                    if h is None or h == 0:
                        st["usb"] = usbp.tile([128, 2, QPIX], bf16, name="usb")
                        usb_out.append(st["usb"])
                    lo = 0 if h is None else h * 512
                    hi = QPIX if h is None else (h + 1) * 512
                    for vt in range(2):
                        nc.scalar.activation(out=st["usb"][:, vt, lo:hi],
                                             in_=st["ups"][vt][:, lo:hi],
                                             func=ACTF.Copy)

                def cv2_units(qq, usb, cvps, ysbp, nlocs=(0, 1)):
                    units = []

                    def u_cv2(nloc, mt):
                        nt = 2 * qq + nloc
                        ps2 = cvps.tile([128, 512], f32, name="cv2ps")
                        for kt in range(2):
                            nc.tensor.matmul(
                                ps2,
                                lhsT=wt2s[kt][:, mt * 128:(mt + 1) * 128],
                                rhs=usb[:, kt, nloc * 512:(nloc + 1) * 512],
                                start=(kt == 0), stop=(kt == 1))
                        ysb = ysbp.tile([128, 512], bf16, name="ysb")
                        nc.scalar.activation(out=ysb, in_=ps2, func=ACTF.Silu,
                                             bias=b2s[mt][:, 0:1], scale=1.0)
                        nc.sync.dma_start(
                            out=y_d[mt * 128:(mt + 1) * 128,
                                    nt * 512:(nt + 1) * 512],
                            in_=ysb)
                    for nloc in nlocs:
                        for mt in range(2):
                            units.append(lambda n=nloc, m=mt: u_cv2(n, m))
                    return units

                def emit_interleaved(primary, secondary):
                    si, n_s = 0, len(secondary)
                    n_p = max(1, len(primary))
                    for k, p in enumerate(primary):
                        p()
                        want = ((k + 1) * n_s) // n_p
                        while si < want:
                            secondary[si]()
                            si += 1
                    while si < n_s:
                        secondary[si]()
                        si += 1

                with tc.tile_pool(name="abcp", bufs=8) as abcp, \
                     tc.tile_pool(name="prodp", bufs=10) as prodp, \
                     tc.tile_pool(name="usbp", bufs=2) as usbp, \
                     tc.tile_pool(name="ysbp", bufs=2) as ysbp:

                    chunk1_units = []
                    if phase >= 2:
                        # startup: A chunk 0 + V rows for quarter-0 half 0
                        su0 = build_chunk_units(0, startup_pool_prods)
                        vb = [lambda: emit_vblock(0), lambda: emit_vblock(1),
                              lambda: emit_vpodd_rows(0, 12)]
                        emit_interleaved(su0, vb)
                        chunk1_units = build_chunk_units(1, startup_pool_prods)
                        chunk1_units += [lambda: emit_vblock(2),
                                         lambda: emit_vpodd_rows(12, 20)]

                    usbs = {}
                    for qq in range(4 if phase >= 3 else 0):
                        # next-quarter build units
                        nxt = []
                        if qq < 3:
                            nxt += build_chunk_units(2 * qq + 2)
                            nxt += build_chunk_units(2 * qq + 3)
                            for ntv in vblock_sched[qq + 1]:
                                nxt.append(lambda ntv=ntv: emit_vblock(ntv))
                            nxt.append(lambda qq=qq: emit_vpodd_seg(qq + 1))

                        # previous quarter's cv2 in its own PSUM window
                        n_pre = 0
                        if qq > 0 and phase >= 4:
                            with tc.tile_pool(name=f"cvps{qq - 1}", bufs=2,
                                              space="PSUM") as cvps:
                                n_pre = min(6, len(nxt))
                                emit_interleaved(
                                    cv2_units(qq - 1, usbs[qq - 1], cvps, ysbp),
                                    nxt[:n_pre])

                        if qq == 2:
                            # last transposes (chunk 7) are in this stream;
                            # free trps banks before quarter 3 so cv2(3) can
                            # overlap quarter-3 accumulation
                            pass

                        with tc.tile_pool(name=f"ups{qq}", bufs=1,
                                          space="PSUM") as upsp:
                            st = {}
                            usb_out = []
                            if qq == 0:
                                prim = (tap_half_units(0, upsp, st, h=0)
                                        + tap_half_units(0, upsp, st, h=1)
                                        + [lambda: u_usb(st, usb_out)])
                                emit_interleaved(prim, chunk1_units + nxt)
                            elif qq < 3:
                                prim = (tap_half_units(qq, upsp, st)
                                        + [lambda: u_usb(st, usb_out)])
                                emit_interleaved(prim, nxt[n_pre:])
                            else:
                                emit_interleaved(
                                    tap_half_units(3, upsp, st, h=0)
                                    + [lambda: u_usb(st, usb_out, h=0)], [])
                                if phase >= 4:
                                    with tc.tile_pool(name="cvps3", bufs=2,
                                                      space="PSUM") as cvps:
                                        emit_interleaved(
                                            tap_half_units(3, upsp, st, h=1)
                                            + [lambda: u_usb(st, usb_out, h=1)],
                                            cv2_units(3, st["usb"], cvps, ysbp,
                                                      nlocs=(0,)))
                                        for u in cv2_units(3, st["usb"], cvps,
                                                           ysbp, nlocs=(1,)):
                                            u()
                                else:
                                    emit_interleaved(
                                        tap_half_units(3, upsp, st, h=1)
                                        + [lambda: u_usb(st, usb_out, h=1)], [])
                            usbs[qq] = usb_out[0] if usb_out else None

    _split_multiwait(nc, mybir)
    return nc


def _prepare(inputs):
    x = np.ascontiguousarray(np.asarray(inputs["x"], np.float32))
    w_cv1 = np.asarray(inputs["w_cv1"], np.float32)
    b_cv1 = np.asarray(inputs["b_cv1"], np.float32)
    w_off = np.asarray(inputs["w_off"], np.float32)
    b_off = np.asarray(inputs["b_off"], np.float32)
    w_cv2 = np.asarray(inputs["w_cv2"], np.float32)
    bn_g = np.asarray(inputs["bn_gamma"], np.float32)
    bn_b = np.asarray(inputs["bn_beta"], np.float32)
    bn_m = np.asarray(inputs["bn_mean"], np.float32)
    bn_v = np.asarray(inputs["bn_var"], np.float32)

    perm_v = _v_perm()
    W1p = w_cv1[perm_v, :]
    b1p = b_cv1[perm_v]

    Wom = w_off @ w_cv1
    bom = w_off @ b_cv1 + b_off
    omp = _om_perm()
    Wom_p = Wom[omp]
    bom_p = bom[omp]

    s = bn_g / np.sqrt(bn_v + BN_EPS)
    W2s = w_cv2 * s[:, None]
    b2f = bn_b - bn_m * s
    W2p = W2s[:, perm_v]

    shared = dict(
        wt1=np.ascontiguousarray(W1p.T),
        wtom=np.ascontiguousarray(Wom_p.T),
        wt2=np.ascontiguousarray(W2p.T).astype(ml_dtypes.bfloat16),
        b1=np.ascontiguousarray(b1p[:, None]),
        b2=np.ascontiguousarray(b2f[:, None]),
        bom=np.ascontiguousarray(bom_p[None, :]),
        idn=np.eye(128, dtype=ml_dtypes.bfloat16),
        onesrow=np.ones((1, 128), np.float32),
    )
    in_maps = []
    for b in range(B):
        m = dict(shared)
        m["x"] = np.ascontiguousarray(x[b].reshape(C1, HW))
        in_maps.append(m)
    return in_maps


def kernel(**inputs):
    from concourse.bass_utils import run_bass_kernel_spmd

    if "nc" not in _cache:
        _cache["nc"] = _build_nc()
    nc = _cache["nc"]
    in_maps = _prepare(inputs)
    res = run_bass_kernel_spmd(nc, in_maps, core_ids=list(range(B)))
    out = np.stack([np.asarray(r["y"], np.float32).reshape(C2, H, W)
                    for r in res.results])
    return out


if __name__ == "__main__":
    rng = np.random.default_rng(0)
    demo = dict(
        x=rng.standard_normal((B, C1, H, W)).astype(np.float32),
        w_cv1=rng.standard_normal((C, C1)).astype(np.float32) / 16,
        b_cv1=(rng.standard_normal((C,)) * 0.1).astype(np.float32),
        w_off=(rng.standard_normal((G * 3 * K, C)) * 0.01).astype(np.float32),
        b_off=(rng.standard_normal((G * 3 * K,)) * 0.01).astype(np.float32),
        w_cv2=rng.standard_normal((C2, C)).astype(np.float32) / 16,
        bn_gamma=rng.uniform(0.5, 1.5, (C2,)).astype(np.float32),
        bn_beta=(rng.standard_normal((C2,)) * 0.1).astype(np.float32),
        bn_mean=(rng.standard_normal((C2,)) * 0.1).astype(np.float32),
        bn_var=rng.uniform(0.5, 1.5, (C2,)).astype(np.float32),
    )
    y = kernel(**demo)
    print("kernel ran, output", y.shape, y.dtype)


# revision 16
# speedup vs baseline: 1.1109x; 1.0480x over previous
"""DCNv4 block (cv1 1x1 -> offset/mask proj -> deformable bilinear sampling
-> cv2 1x1 -> BN -> SiLU) as a Bass/Tile kernel for Trainium2.

Strategy
--------
Data-parallel over batch: each of the 8 NeuronCores processes one image.

The deformable sampling is reformulated gather-free: with |off| < 1 the
bilinear sample of kernel point k at (h+kh+off_h, w+kw+off_w) equals
  sum_{i,j in {-1,0,1}} tent(off_h - i) * tent(off_w - j) * V[h+kh+i, w+kw+j]
with tent(t) = max(0, 1-|t|).  Merging all 9 kernel points over absolute
displacements e=(eh,ew) in [-2,2]^2 gives 25 "taps":
  out[p,g,:] = sum_e A_e[p,g] * Vpad[p+e, g, :]
  A_e[p,g]   = sum_k mask_k * tent(off_h - (eh-kh)) * tent(off_w - (ew-kw))
Out-of-image corners are handled exactly by zero-padding Vpad (the reference
drops those corners).

Engine mapping:
 - PE: cv1 / offset-projection / cv2 matmuls (float32r), A^T transposes,
   and the 25-term tap accumulation as identity-weight matmuls accumulating
   into PSUM (f32 accumulation).
 - DVE: tent products, A scatter-build, most per-tap elementwise A*V
   products (both 128-channel tiles fused into one [128,2048] op via a
   stride-0 repeat on the A operand).
 - ACT: tents read the offset projection directly from PSUM; cv1 bias is
   applied by the PSUM->SBUF copy's per-partition bias; BN+SiLU epilogue.
 - GPSIMD: 7 of 25 tap products per quarter, 2 A-build products, memsets
   (trimmed to only the A slots not freshly written by the (0,0) scatter,
   which is a copy instead of an accumulate).
 - DMA: a replicating access pattern broadcasts per-group tap maps A_e[g,:]
   (16 partitions) to all 128 partitions (V channels are laid out g-major,
   partition j -> group j//8, so one broadcast serves both channel tiles).

BN is folded into cv2 on the host; the offset projection is folded through
cv1 on the host so offsets are computed from x at full precision independent
of V's bf16 rounding.  The output DMA is bf16 (cast to f32 on host).
"""

import sys
import numpy as np

if "/opt/trn_rl_repo" not in sys.path:
    sys.path.insert(0, "/opt/trn_rl_repo")

import ml_dtypes

B, C1, C2, H, W = 8, 256, 256, 64, 64
C = 256
G = 16
Cg = 16
K = 9
HW = H * W           # 4096
PW = W + 4           # 68
PH = H + 4
BN_EPS = 1e-5
TPAD = 32            # taps padded to 32 so (t, g) blocks are 128-aligned
OM = 432             # offset/mask projection width (3*K*G)

_cache = {}


def _v_perm():
    # vtile vt, partition j  ->  original channel g*16 + c
    perm = []
    for vt in range(2):
        for j in range(128):
            g = j // 8
            c = vt * 8 + (j % 8)
            perm.append(g * Cg + c)
    return np.array(perm, np.int64)


def _om_perm():
    # om channel r (0..431) -> original w_off row
    rows = np.zeros(432, np.int64)
    for r in range(144):
        k, g = r // 16, r % 16
        rows[r] = g * 27 + 2 * k            # off_h (dh)
        rows[144 + r] = g * 27 + 2 * k + 1  # off_w (dw)
        rows[288 + r] = g * 27 + 18 + k     # mask
    return rows


def _split_multiwait(nc, mybir, max_waits=1):
    """walrus in this container rejects >1 sem wait on one instruction;
    split extras onto preceding same-engine NoOps (equivalent ordering)."""
    for f in nc.m.functions:
        for bb in f.blocks:
            out = []
            for inst in bb.instructions:
                si = inst.sync_info
                if si is not None and len(si.on_wait) > max_waits:
                    waits = list(si.on_wait)
                    for w in waits[:-max_waits]:
                        nop = mybir.InstNoOp(
                            name=f"I-nopw{nc.next_id()}", ins=[], outs=[])
                        nop.engine = inst.engine
                        nop.sync_info = mybir.SyncInfo(on_wait=[w], on_update=[])
                        nc.register_instruction(nop)
                        out.append(nop)
                    si.on_wait = waits[-max_waits:]
                out.append(inst)
            bb.instructions = out


def _build_nc(phase=99, pace=0.85, n_pre=6, pool_taps_q=((2, 5, 9, 12, 15, 19, 22),) * 3 + ((2, 5, 12, 15, 19, 22),),
              startup_pool_prods=((1, 1), (2, 2), (0, 1))):
    import concourse.bass as bass
    import concourse.mybir as mybir
    import concourse.tile as tile

    f32 = mybir.dt.float32
    f32r = mybir.dt.float32r
    bf16 = mybir.dt.bfloat16
    ALU = mybir.AluOpType
    ACTF = mybir.ActivationFunctionType

    nc = bass.Bass()

    x_d = nc.dram_tensor("x", [C1, HW], f32r, kind="ExternalInput")
    wt1_d = nc.dram_tensor("wt1", [C1, 256], f32r, kind="ExternalInput")
    wtom_d = nc.dram_tensor("wtom", [C1, OM], f32r, kind="ExternalInput")
    wt2_d = nc.dram_tensor("wt2", [C, C2], bf16, kind="ExternalInput")
    b1_d = nc.dram_tensor("b1", [C, 1], f32, kind="ExternalInput")
    b2_d = nc.dram_tensor("b2", [C2, 1], f32, kind="ExternalInput")
    bom_d = nc.dram_tensor("bom", [1, OM], f32r, kind="ExternalInput")
    idn_d = nc.dram_tensor("idn", [128, 128], bf16, kind="ExternalInput")
    ones_d = nc.dram_tensor("onesrow", [1, 128], f32r, kind="ExternalInput")
    y_d = nc.dram_tensor("y", [C2, HW], bf16, kind="ExternalOutput")

    with tile.TileContext(nc) as tc:
        with tc.tile_pool(name="persist", bufs=1) as persist:

            # ---- persistent tiles ----
            wt1s = [persist.tile([128, 256], f32r, name=f"wt1_{i}") for i in range(2)]
            wtoms = [persist.tile([128, OM], f32r, name=f"wtom_{i}") for i in range(2)]
            wt2s = [persist.tile([128, 256], bf16, name=f"wt2_{i}") for i in range(2)]
            b1s = [persist.tile([128, 1], f32, name=f"b1_{i}") for i in range(2)]
            bom1 = persist.tile([1, OM], f32r, name="bom1")
            b2s = [persist.tile([128, 1], f32, name=f"b2_{i}") for i in range(2)]
            ones = persist.tile([1, 128], f32r, name="ones")
            idn = persist.tile([128, 128], bf16, name="idn")
            # V and its 1-col-shifted copy, both channel tiles fused:
            # [128 part, vt, padded rows, padded cols]
            vpad = persist.tile([128, 2, PH, PW], bf16, name="vpad")
            vpodd = persist.tile([128, 2, PH * PW], bf16, name="vpodd")
            atile = [persist.tile([128, HW], bf16, name=f"atile_{i}") for i in range(4)]

            for i in range(2):
                nc.sync.dma_start(out=wtoms[i], in_=wtom_d[i * 128:(i + 1) * 128, :])
            nc.sync.dma_start(out=bom1, in_=bom_d[:, :])
            nc.sync.dma_start(out=ones, in_=ones_d[:, :])

            # zero the pad ring of Vpad (interior written by cv1)
            nc.vector.memset(vpad[:, :, 0:2, :], 0.0)
            nc.vector.memset(vpad[:, :, PH - 2:PH, :], 0.0)
            nc.vector.memset(vpad[:, :, 2:PH - 2, 0:2], 0.0)
            nc.vector.memset(vpad[:, :, 2:PH - 2, PW - 4:PW], 0.0)

            with tc.tile_pool(name="build", bufs=1) as bpool, \
                 tc.tile_pool(name="tbuf", bufs=2) as tbuf, \
                 tc.tile_pool(name="ombuf", bufs=2) as ombuf, \
                 tc.tile_pool(name="prodab", bufs=4) as prodab, \
                 tc.tile_pool(name="atbuf", bufs=2) as atbuf, \
                 tc.tile_pool(name="psB", bufs=2, space="PSUM") as psB:

                from contextlib import ExitStack
                trps_stack = ExitStack()
                trps = trps_stack.enter_context(
                    tc.tile_pool(name="trps", bufs=2, space="PSUM"))

                xs = [bpool.tile([128, HW], f32r, name=f"xs_{i}") for i in range(2)]

                for i in range(2):
                    nc.sync.dma_start(
                        out=xs[i][:, 0:1024],
                        in_=x_d[i * 128:(i + 1) * 128, 0:1024])
                nc.sync.dma_start(out=idn, in_=idn_d[:, :])
                for i in range(2):
                    nc.sync.dma_start(out=wt1s[i], in_=wt1_d[i * 128:(i + 1) * 128, :])
                    nc.sync.dma_start(out=b1s[i], in_=b1_d[i * 128:(i + 1) * 128, :])
                for i in range(2):
                    nc.sync.dma_start(
                        out=xs[i][:, 1024:HW],
                        in_=x_d[i * 128:(i + 1) * 128, 1024:HW])
                for i in range(2):
                    nc.sync.dma_start(out=wt2s[i], in_=wt2_d[i * 128:(i + 1) * 128, :])
                    nc.sync.dma_start(out=b2s[i], in_=b2_d[i * 128:(i + 1) * 128, :])

                taps = [(eh, ew) for eh in range(-2, 3) for ew in range(-2, 3)]
                QPIX = 1024          # pixels per tap quarter (16 image rows)
                cpt = 4

                def emit_vblock(nt):
                    # cv1 rows nt*8..nt*8+8 for both channel tiles; bias is
                    # applied by the PSUM->SBUF activation copy
                    for mt in range(2):
                        ps = psB.tile([128, 512], f32, name="omm")
                        for kt in range(2):
                            nc.tensor.matmul(
                                ps, lhsT=wt1s[kt][:, mt * 128:(mt + 1) * 128],
                                rhs=xs[kt][:, nt * 512:(nt + 1) * 512],
                                start=(kt == 0), stop=(kt == 1))
                        r0v = nt * 8
                        nc.scalar.activation(
                            out=vpad[:, mt, 2 + r0v:2 + r0v + 8, 2:2 + W],
                            in_=ps[:].rearrange("p (r c) -> p r c", c=W),
                            func=ACTF.Identity, bias=b1s[mt][:, 0:1], scale=1.0)

                def emit_vpodd_rows(lo_row, hi_row):
                    # vpodd[p, v, i] = vpad[p, v, i+1]; segment rows chosen so
                    # consecutive segments do not overlap (no WAR with the
                    # previous quarter's odd-tap reads)
                    lo = lo_row * PW
                    hi = min(hi_row * PW, PH * PW) - 1
                    vflat = vpad[:].rearrange("p v a b -> p v (a b)")
                    nc.scalar.activation(out=vpodd[:, :, lo:hi],
                                         in_=vflat[:, :, lo + 1:hi + 1],
                                         func=ACTF.Copy)

                def emit_vpodd_seg(qq):
                    emit_vpodd_rows(qq * 16 + (4 if qq else 0), qq * 16 + 20)

                vblock_sched = {0: [0, 1, 2], 1: [3, 4], 2: [5, 6], 3: [7]}

                def build_chunk_units(chk, pool_prods=((1, 1), (2, 2))):
                    """Closure units for A-map chunk chk (4 pixel-tiles)."""
                    st = {}
                    units = []

                    def u_om(pi):
                        if pi == 0:
                            st["om_t"] = ombuf.tile([128, cpt, OM], bf16,
                                                    name="om_t")
                        pt = chk * cpt + pi
                        ps = psB.tile([128, OM], f32, name="omm")
                        for kt in range(2):
                            nc.tensor.matmul(
                                ps, lhsT=xs[kt][:, pt * 128:(pt + 1) * 128],
                                rhs=wtoms[kt][:, :],
                                start=(kt == 0), stop=False)
                        nc.tensor.matmul(
                            ps, lhsT=ones[0:1, :],
                            rhs=bom1[0:1, :], start=False, stop=True)
                        nc.scalar.activation(out=st["om_t"][:, pi, :], in_=ps,
                                             func=ACTF.Copy)
                    for pi in range(cpt):
                        units.append(lambda pi=pi: u_om(pi))

                    def u_tents():
                        om_t = st["om_t"]
                        oh = om_t[:, :, 0:144]
                        ow = om_t[:, :, 144:288]
                        st["mbf"] = om_t[:, :, 288:432]
                        th = [tbuf.tile([128, cpt, 144], bf16, name=f"th_{i}")
                              for i in range(3)]
                        tw = [tbuf.tile([128, cpt, 144], bf16, name=f"tw_{i}")
                              for i in range(3)]
                        st["th"], st["tw"] = th, tw
                        # tents (bf16): index 0,1,2 <-> i=-1,0,+1
                        # t(-1)=relu(-o); t(+1)=relu(o); slot1 holds NEGATED
                        # t(0): |o|-1 = relu(o)+relu(-o)-1.
                        nc.scalar.activation(out=th[2], in_=oh, func=ACTF.Relu)
                        nc.scalar.activation(out=th[0], in_=oh, func=ACTF.Relu,
                                             scale=-1.0)
                        nc.scalar.activation(out=tw[2], in_=ow, func=ACTF.Relu)
                        nc.scalar.activation(out=tw[0], in_=ow, func=ACTF.Relu,
                                             scale=-1.0)
                        nc.vector.scalar_tensor_tensor(
                            out=th[1], in0=th[2], scalar=-1.0, in1=th[0],
                            op0=ALU.add, op1=ALU.add)
                        nc.vector.scalar_tensor_tensor(
                            out=tw[1], in0=tw[2], scalar=-1.0, in1=tw[0],
                            op0=ALU.add, op1=ALU.add)
                    units.append(u_tents)

                    def u_masks():
                        for i in range(3):
                            nc.vector.tensor_tensor(out=st["th"][i],
                                                    in0=st["th"][i],
                                                    in1=st["mbf"], op=ALU.mult)
                    units.append(u_masks)

                    def u_memset():
                        # (0,0) scatter is a fresh write covering taps r<3,s<3;
                        # zero only the complement
                        at = atbuf.tile([128, cpt, TPAD * 16], bf16, name="at")
                        st["at"] = at
                        a_ap = at[:, :, :]
                        nc.gpsimd.memset(bass.AP(
                            a_ap.tensor, a_ap.offset + 3 * 16,
                            [[cpt * TPAD * 16, 128], [TPAD * 16, cpt],
                             [5 * 16, 3], [1, 32]]), 0.0)
                        nc.gpsimd.memset(bass.AP(
                            a_ap.tensor, a_ap.offset + 15 * 16,
                            [[cpt * TPAD * 16, 128], [TPAD * 16, cpt],
                             [1, 160]]), 0.0)
                    units.append(u_memset)

                    def u_prod(i, j):
                        peng = nc.gpsimd if (i, j) in pool_prods else nc.vector
                        prod = prodab.tile([128, cpt, 144], bf16, name="prodb")
                        peng.tensor_tensor(out=prod, in0=st["th"][i],
                                           in1=st["tw"][j], op=ALU.mult)
                        a_ap = st["at"][:, :, :]
                        o_ap = bass.AP(
                            a_ap.tensor, a_ap.offset + (i * 5 + j) * 16,
                            [[cpt * TPAD * 16, 128], [TPAD * 16, cpt],
                             [5 * 16, 3], [1, 48]])
                        p_ap = prod[:, :, :]
                        i_ap = bass.AP(
                            p_ap.tensor, p_ap.offset,
                            [[cpt * 144, 128], [144, cpt], [48, 3], [1, 48]])
                        if (i, j) == (0, 0):
                            nc.vector.tensor_copy(out=o_ap, in_=i_ap)
                        else:
                            sop = ALU.subtract if (i == 1) != (j == 1) else ALU.add
                            nc.vector.tensor_tensor(out=o_ap, in0=o_ap,
                                                    in1=i_ap, op=sop)
                    for i in range(3):
                        for j in range(3):
                            units.append(lambda i=i, j=j: u_prod(i, j))

                    def u_transpose(tb):
                        tps = trps.tile([128, 512], bf16, name="tr")
                        for s in range(4):
                            nc.tensor.transpose(
                                tps[:, s * 128:(s + 1) * 128],
                                st["at"][:, s, tb * 128:(tb + 1) * 128],
                                idn[:, :])
                        col = chk * cpt * 128
                        nc.scalar.activation(
                            out=atile[tb][:, col:col + 512], in_=tps,
                            func=ACTF.Copy)
                    for tb in range(4):
                        units.append(lambda tb=tb: u_transpose(tb))
                    return units

                def tap_half_units(qq, upsp, st, h=None):
                    """Tap units for quarter qq; h=None -> full 1024-pixel
                    quarter, h=0/1 -> 512-pixel half (own abc DMAs and
                    per-column-region PSUM chains)."""
                    pool_taps = pool_taps_q[qq]
                    units = []
                    r0 = qq * 16
                    order = ([t for t in range(len(taps)) if t in pool_taps]
                             + [t for t in range(len(taps)) if t not in pool_taps])

                    def u_tap(t, seq, h):
                        eh, ew = taps[t]
                        if seq == 0 and (h is None or h == 0):
                            st["ups"] = [upsp.tile([128, QPIX], f32,
                                                   name=f"ups_{v}")
                                         for v in range(2)]
                        npix = QPIX if h is None else 512
                        coff = 0 if h is None else h * 512
                        rows = npix // W
                        tb, ts = t // 8, t % 8
                        abc = abcp.tile([128, npix], bf16, name="abc")
                        a_ap2 = atile[tb][:, :]
                        sap = bass.AP(
                            a_ap2.tensor,
                            a_ap2.offset + ts * 16 * HW + qq * QPIX + coff,
                            [[HW, 16], [0, 8], [1, npix]])
                        nc.sync.dma_start(out=abc, in_=sap)
                        # A operand repeated over both channel tiles via a
                        # stride-0 dim
                        b_ap = abc[:, :]
                        abc4 = bass.AP(
                            b_ap.tensor, b_ap.offset,
                            [[npix, 128], [0, 2], [W, rows], [1, W]])
                        rbase = 2 + r0 + (0 if h is None else h * 8) + eh
                        if ew % 2 == 0:
                            win = vpad[:, :, rbase:rbase + rows,
                                       2 + ew:2 + ew + W]
                        else:
                            vp4 = vpodd[:].rearrange("p v (a b) -> p v a b",
                                                     b=PW)
                            win = vp4[:, :, rbase:rbase + rows,
                                      1 + ew:1 + ew + W]
                        eng = nc.gpsimd if t in pool_taps else nc.vector
                        pr = prodp.tile([128, 2, rows, W], bf16, name="tp")
                        eng.tensor_tensor(out=pr, in0=win, in1=abc4,
                                          op=ALU.mult)
                        prf = pr[:].rearrange("p v a b -> p (v a b)")
                        for vt in range(2):
                            for nb in range(npix // 512):
                                nc.tensor.matmul(
                                    st["ups"][vt][:, coff + nb * 512:
                                                  coff + (nb + 1) * 512],
                                    lhsT=idn[:, :],
                                    rhs=prf[:, vt * npix + nb * 512:
                                            vt * npix + (nb + 1) * 512],
                                    start=(seq == 0),
                                    stop=(seq == len(taps) - 1))
                    for seq, t in enumerate(order):
                        units.append(lambda t=t, seq=seq, h=h: u_tap(t, seq, h))
                    return units

                def u_usb(st, usb_out, h=None):
                    if h is None or h == 0:
                        st["usb"] = usbp.tile([128, 2, QPIX], bf16, name="usb")
                        usb_out.append(st["usb"])
                    lo = 0 if h is None else h * 512
                    hi = QPIX if h is None else (h + 1) * 512
                    for vt in range(2):
                        nc.scalar.activation(out=st["usb"][:, vt, lo:hi],
                                             in_=st["ups"][vt][:, lo:hi],
                                             func=ACTF.Copy)

                def cv2_units(qq, usb, cvps, ysbp, nlocs=(0, 1)):
                    units = []
                    ysbs = {}

                    def u_cv2(nloc, mt):
                        if (nloc, mt % 2) not in ysbs and nloc == 0:
                            pass
                        ps2 = cvps.tile([128, 512], f32, name="cv2ps")
                        for kt in range(2):
                            nc.tensor.matmul(
                                ps2,
                                lhsT=wt2s[kt][:, mt * 128:(mt + 1) * 128],
                                rhs=usb[:, kt, nloc * 512:(nloc + 1) * 512],
                                start=(kt == 0), stop=(kt == 1))
                        if mt not in ysbs:
                            ysbs[mt] = ysbp.tile([128, QPIX], bf16,
                                                 name=f"ysb{mt}")
                        ysb = ysbs[mt]
                        nc.scalar.activation(out=ysb[:, nloc * 512:
                                                     (nloc + 1) * 512],
                                             in_=ps2, func=ACTF.Silu,
                                             bias=b2s[mt][:, 0:1], scale=1.0)
                        if nloc == 1:
                            nc.scalar.dma_start(
                                out=y_d[mt * 128:(mt + 1) * 128,
                                        qq * QPIX:(qq + 1) * QPIX],
                                in_=ysb)
                    for nloc in nlocs:
                        for mt in range(2):
                            units.append(lambda n=nloc, m=mt: u_cv2(n, m))
                    return units

                def emit_interleaved(primary, secondary, pace=1.0):
                    # pace < 1 front-loads: secondary exhausted after that
                    # fraction of primary
                    si, n_s = 0, len(secondary)
                    n_p = max(1, int(len(primary) * pace))
                    for k, p in enumerate(primary):
                        p()
                        want = min(n_s, ((k + 1) * n_s) // n_p)
                        while si < want:
                            secondary[si]()
                            si += 1
                    while si < n_s:
                        secondary[si]()
                        si += 1

                with tc.tile_pool(name="abcp", bufs=6) as abcp, \
                     tc.tile_pool(name="prodp", bufs=8) as prodp, \
                     tc.tile_pool(name="usbp", bufs=2) as usbp, \
                     tc.tile_pool(name="ysbp", bufs=2) as ysbp:

                    if phase >= 2:
                        # startup: A chunks 0,1 pipelined pairwise + V rows
                        # for quarter 0
                        c0 = build_chunk_units(0, startup_pool_prods)
                        c1 = build_chunk_units(1, startup_pool_prods)
                        zipped = [u for pair in zip(c0, c1) for u in pair]
                        vb = [lambda: emit_vblock(0), lambda: emit_vblock(1),
                              lambda: emit_vblock(2),
                              lambda: emit_vpodd_rows(0, 20)]
                        emit_interleaved(zipped, vb, pace=0.6)

                    usbs = {}
                    for qq in range(4 if phase >= 3 else 0):
                        # next-quarter build units
                        nxt = []
                        if qq < 3:
                            nxt += build_chunk_units(2 * qq + 2)
                            nxt += build_chunk_units(2 * qq + 3)
                            for ntv in vblock_sched[qq + 1]:
                                nxt.append(lambda ntv=ntv: emit_vblock(ntv))
                            nxt.append(lambda qq=qq: emit_vpodd_seg(qq + 1))

                        # previous quarter's cv2 in its own PSUM window
                        n_pre = 0
                        if qq > 0 and phase >= 4:
                            with tc.tile_pool(name=f"cvps{qq - 1}", bufs=2,
                                              space="PSUM") as cvps:
                                n_pre = min(n_pre, len(nxt))
                                emit_interleaved(
                                    cv2_units(qq - 1, usbs[qq - 1], cvps, ysbp),
                                    nxt[:n_pre])

                        if qq == 3:
                            # chunk 7's transposes (emitted in quarter 2's
                            # stream) are the last trps use; free its banks
                            trps_stack.close()

                        with tc.tile_pool(name=f"ups{qq}", bufs=1,
                                          space="PSUM") as upsp:
                            st = {}
                            usb_out = []
                            prim = (tap_half_units(qq, upsp, st)
                                    + [lambda: u_usb(st, usb_out)])
                            emit_interleaved(prim, nxt[n_pre:], pace=pace)
                            usbs[qq] = usb_out[0]

                    if phase >= 4:
                        with tc.tile_pool(name="cvps3", bufs=2,
                                          space="PSUM") as cvps:
                            for u in cv2_units(3, usbs[3], cvps, ysbp):
                                u()

    _split_multiwait(nc, mybir)
    return nc


def _prepare(inputs):
    x = np.ascontiguousarray(np.asarray(inputs["x"], np.float32))
    w_cv1 = np.asarray(inputs["w_cv1"], np.float32)
    b_cv1 = np.asarray(inputs["b_cv1"], np.float32)
    w_off = np.asarray(inputs["w_off"], np.float32)
    b_off = np.asarray(inputs["b_off"], np.float32)
    w_cv2 = np.asarray(inputs["w_cv2"], np.float32)
    bn_g = np.asarray(inputs["bn_gamma"], np.float32)
    bn_b = np.asarray(inputs["bn_beta"], np.float32)
    bn_m = np.asarray(inputs["bn_mean"], np.float32)
    bn_v = np.asarray(inputs["bn_var"], np.float32)

    perm_v = _v_perm()
    W1p = w_cv1[perm_v, :]
    b1p = b_cv1[perm_v]

    Wom = w_off @ w_cv1
    bom = w_off @ b_cv1 + b_off
    omp = _om_perm()
    Wom_p = Wom[omp]
    bom_p = bom[omp]

    s = bn_g / np.sqrt(bn_v + BN_EPS)
    W2s = w_cv2 * s[:, None]
    b2f = bn_b - bn_m * s
    W2p = W2s[:, perm_v]

    shared = dict(
        wt1=np.ascontiguousarray(W1p.T),
        wtom=np.ascontiguousarray(Wom_p.T),
        wt2=np.ascontiguousarray(W2p.T).astype(ml_dtypes.bfloat16),
        b1=np.ascontiguousarray(b1p[:, None]),
        b2=np.ascontiguousarray(b2f[:, None]),
        bom=np.ascontiguousarray(bom_p[None, :]),
        idn=np.eye(128, dtype=ml_dtypes.bfloat16),
        onesrow=np.ones((1, 128), np.float32),
    )
    in_maps = []
    for b in range(B):
        m = dict(shared)
        m["x"] = np.ascontiguousarray(x[b].reshape(C1, HW))
        in_maps.append(m)
    return in_maps


def kernel(**inputs):
    from concourse.bass_utils import run_bass_kernel_spmd

    if "nc" not in _cache:
        _cache["nc"] = _build_nc()
    nc = _cache["nc"]
    in_maps = _prepare(inputs)
    res = run_bass_kernel_spmd(nc, in_maps, core_ids=list(range(B)))
    out = np.stack([np.asarray(r["y"], np.float32).reshape(C2, H, W)
                    for r in res.results])
    return out


if __name__ == "__main__":
    rng = np.random.default_rng(0)
    demo = dict(
        x=rng.standard_normal((B, C1, H, W)).astype(np.float32),
        w_cv1=rng.standard_normal((C, C1)).astype(np.float32) / 16,
        b_cv1=(rng.standard_normal((C,)) * 0.1).astype(np.float32),
        w_off=(rng.standard_normal((G * 3 * K, C)) * 0.01).astype(np.float32),
        b_off=(rng.standard_normal((G * 3 * K,)) * 0.01).astype(np.float32),
        w_cv2=rng.standard_normal((C2, C)).astype(np.float32) / 16,
        bn_gamma=rng.uniform(0.5, 1.5, (C2,)).astype(np.float32),
        bn_beta=(rng.standard_normal((C2,)) * 0.1).astype(np.float32),
        bn_mean=(rng.standard_normal((C2,)) * 0.1).astype(np.float32),
        bn_var=rng.uniform(0.5, 1.5, (C2,)).astype(np.float32),
    )
    y = kernel(**demo)
    print("kernel ran, output", y.shape, y.dtype)


# revision 28
# speedup vs baseline: 1.1141x; 1.0029x over previous
"""DCNv4 block (cv1 1x1 -> offset/mask proj -> deformable bilinear sampling
-> cv2 1x1 -> BN -> SiLU) as a Bass/Tile kernel for Trainium2.

Strategy
--------
Data-parallel over batch: each of the 8 NeuronCores processes one image.

The deformable sampling is reformulated gather-free: with |off| < 1 the
bilinear sample of kernel point k at (h+kh+off_h, w+kw+off_w) equals
  sum_{i,j in {-1,0,1}} tent(off_h - i) * tent(off_w - j) * V[h+kh+i, w+kw+j]
with tent(t) = max(0, 1-|t|).  Merging all 9 kernel points over absolute
displacements e=(eh,ew) in [-2,2]^2 gives 25 "taps":
  out[p,g,:] = sum_e A_e[p,g] * Vpad[p+e, g, :]
  A_e[p,g]   = sum_k mask_k * tent(off_h - (eh-kh)) * tent(off_w - (ew-kw))
Out-of-image corners are handled exactly by zero-padding Vpad (the reference
drops those corners).

Engine mapping:
 - PE: cv1 / offset-projection / cv2 matmuls (float32r), A^T transposes,
   and the 25-term tap accumulation as identity-weight matmuls accumulating
   into PSUM (f32 accumulation).
 - DVE: tent products, A scatter-build, most per-tap elementwise A*V
   products (both 128-channel tiles fused into one [128,2048] op via a
   stride-0 repeat on the A operand).
 - ACT: tents read the offset projection directly from PSUM; cv1 bias is
   applied by the PSUM->SBUF copy's per-partition bias; BN+SiLU epilogue.
 - GPSIMD: 7 of 25 tap products per quarter, 2 A-build products, memsets
   (trimmed to only the A slots not freshly written by the (0,0) scatter,
   which is a copy instead of an accumulate).
 - DMA: a replicating access pattern broadcasts per-group tap maps A_e[g,:]
   (16 partitions) to all 128 partitions (V channels are laid out g-major,
   partition j -> group j//8, so one broadcast serves both channel tiles).

BN is folded into cv2 on the host; the offset projection is folded through
cv1 on the host so offsets are computed from x at full precision independent
of V's bf16 rounding.  The output DMA is bf16 (cast to f32 on host).
"""

import sys
import numpy as np

if "/opt/trn_rl_repo" not in sys.path:
    sys.path.insert(0, "/opt/trn_rl_repo")

import ml_dtypes

B, C1, C2, H, W = 8, 256, 256, 64, 64
C = 256
G = 16
Cg = 16
K = 9
HW = H * W           # 4096
PW = W + 4           # 68
PH = H + 4
BN_EPS = 1e-5
TPAD = 32            # taps padded to 32 so (t, g) blocks are 128-aligned
OM = 432             # offset/mask projection width (3*K*G)

_cache = {}


def _v_perm():
    # vtile vt, partition j  ->  original channel g*16 + c
    perm = []
    for vt in range(2):
        for j in range(128):
            g = j // 8
            c = vt * 8 + (j % 8)
            perm.append(g * Cg + c)
    return np.array(perm, np.int64)


def _om_perm():
    # om channel r (0..431) -> original w_off row
    rows = np.zeros(432, np.int64)
    for r in range(144):
        k, g = r // 16, r % 16
        rows[r] = g * 27 + 2 * k            # off_h (dh)
        rows[144 + r] = g * 27 + 2 * k + 1  # off_w (dw)
        rows[288 + r] = g * 27 + 18 + k     # mask
    return rows


def _split_multiwait(nc, mybir, max_waits=1):
    """walrus in this container rejects >1 sem wait on one instruction;
    split extras onto preceding same-engine NoOps (equivalent ordering)."""
    for f in nc.m.functions:
        for bb in f.blocks:
            out = []
            for inst in bb.instructions:
                si = inst.sync_info
                if si is not None and len(si.on_wait) > max_waits:
                    waits = list(si.on_wait)
                    for w in waits[:-max_waits]:
                        nop = mybir.InstNoOp(
                            name=f"I-nopw{nc.next_id()}", ins=[], outs=[])
                        nop.engine = inst.engine
                        nop.sync_info = mybir.SyncInfo(on_wait=[w], on_update=[])
                        nc.register_instruction(nop)
                        out.append(nop)
                    si.on_wait = waits[-max_waits:]
                out.append(inst)
            bb.instructions = out


def _build_nc(phase=99, pace=0.85, n_pre=6, n_prefetch=0, pool_taps_q=((2, 5, 9, 12, 15, 19, 22),) * 3 + ((2, 5, 12, 15, 19, 22),),
              startup_pool_prods=((1, 1), (2, 2), (0, 1))):
    import concourse.bass as bass
    import concourse.mybir as mybir
    import concourse.tile as tile

    f32 = mybir.dt.float32
    f32r = mybir.dt.float32r
    bf16 = mybir.dt.bfloat16
    ALU = mybir.AluOpType
    ACTF = mybir.ActivationFunctionType

    nc = bass.Bass()

    x_d = nc.dram_tensor("x", [C1, HW], f32r, kind="ExternalInput")
    wt1_d = nc.dram_tensor("wt1", [C1, 256], f32r, kind="ExternalInput")
    wtom_d = nc.dram_tensor("wtom", [C1, OM], f32r, kind="ExternalInput")
    wt2_d = nc.dram_tensor("wt2", [C, C2], bf16, kind="ExternalInput")
    b1_d = nc.dram_tensor("b1", [C, 1], f32, kind="ExternalInput")
    b2_d = nc.dram_tensor("b2", [C2, 1], f32, kind="ExternalInput")
    bom_d = nc.dram_tensor("bom", [1, OM], f32r, kind="ExternalInput")
    idn_d = nc.dram_tensor("idn", [128, 128], bf16, kind="ExternalInput")
    ones_d = nc.dram_tensor("onesrow", [1, 128], f32r, kind="ExternalInput")
    y_d = nc.dram_tensor("y", [C2, HW], bf16, kind="ExternalOutput")

    with tile.TileContext(nc) as tc:
        with tc.tile_pool(name="persist", bufs=1) as persist:

            # ---- persistent tiles ----
            wt1s = [persist.tile([128, 256], f32r, name=f"wt1_{i}") for i in range(2)]
            wtoms = [persist.tile([128, OM], f32r, name=f"wtom_{i}") for i in range(2)]
            wt2s = [persist.tile([128, 256], bf16, name=f"wt2_{i}") for i in range(2)]
            b1s = [persist.tile([128, 1], f32, name=f"b1_{i}") for i in range(2)]
            bom1 = persist.tile([1, OM], f32r, name="bom1")
            b2s = [persist.tile([128, 1], f32, name=f"b2_{i}") for i in range(2)]
            ones = persist.tile([1, 128], f32r, name="ones")
            idn = persist.tile([128, 128], bf16, name="idn")
            # V and its 1-col-shifted copy, both channel tiles fused:
            # [128 part, vt, padded rows, padded cols]
            vpad = persist.tile([128, 2, PH, PW], bf16, name="vpad")
            vpodd = persist.tile([128, 2, PH * PW], bf16, name="vpodd")
            atile = [persist.tile([128, HW], bf16, name=f"atile_{i}") for i in range(4)]

            for i in range(2):
                nc.sync.dma_start(out=wtoms[i], in_=wtom_d[i * 128:(i + 1) * 128, :])
            nc.sync.dma_start(out=bom1, in_=bom_d[:, :])
            nc.sync.dma_start(out=ones, in_=ones_d[:, :])

            # zero the pad ring of Vpad (interior written by cv1)
            nc.vector.memset(vpad[:, :, 0:2, :], 0.0)
            nc.vector.memset(vpad[:, :, PH - 2:PH, :], 0.0)
            nc.vector.memset(vpad[:, :, 2:PH - 2, 0:2], 0.0)
            nc.vector.memset(vpad[:, :, 2:PH - 2, PW - 4:PW], 0.0)

            with tc.tile_pool(name="build", bufs=1) as bpool, \
                 tc.tile_pool(name="tbuf", bufs=2) as tbuf, \
                 tc.tile_pool(name="ombuf", bufs=2) as ombuf, \
                 tc.tile_pool(name="prodab", bufs=4) as prodab, \
                 tc.tile_pool(name="atbuf", bufs=2) as atbuf, \
                 tc.tile_pool(name="psB", bufs=2, space="PSUM") as psB:

                from contextlib import ExitStack
                trps_stack = ExitStack()
                trps = trps_stack.enter_context(
                    tc.tile_pool(name="trps", bufs=2, space="PSUM"))

                xs = [bpool.tile([128, HW], f32r, name=f"xs_{i}") for i in range(2)]

                for i in range(2):
                    nc.sync.dma_start(
                        out=xs[i][:, 0:1024],
                        in_=x_d[i * 128:(i + 1) * 128, 0:1024])
                nc.sync.dma_start(out=idn, in_=idn_d[:, :])
                for i in range(2):
                    nc.sync.dma_start(out=wt1s[i], in_=wt1_d[i * 128:(i + 1) * 128, :])
                    nc.sync.dma_start(out=b1s[i], in_=b1_d[i * 128:(i + 1) * 128, :])
                for i in range(2):
                    nc.sync.dma_start(
                        out=xs[i][:, 1024:HW],
                        in_=x_d[i * 128:(i + 1) * 128, 1024:HW])
                for i in range(2):
                    nc.sync.dma_start(out=wt2s[i], in_=wt2_d[i * 128:(i + 1) * 128, :])
                    nc.sync.dma_start(out=b2s[i], in_=b2_d[i * 128:(i + 1) * 128, :])

                taps = [(eh, ew) for eh in range(-2, 3) for ew in range(-2, 3)]
                QPIX = 1024          # pixels per tap quarter (16 image rows)
                cpt = 4

                def emit_vblock(nt):
                    # cv1 rows nt*8..nt*8+8 for both channel tiles; bias is
                    # applied by the PSUM->SBUF activation copy
                    for mt in range(2):
                        ps = psB.tile([128, 512], f32, name="omm")
                        for kt in range(2):
                            nc.tensor.matmul(
                                ps, lhsT=wt1s[kt][:, mt * 128:(mt + 1) * 128],
                                rhs=xs[kt][:, nt * 512:(nt + 1) * 512],
                                start=(kt == 0), stop=(kt == 1))
                        r0v = nt * 8
                        nc.scalar.activation(
                            out=vpad[:, mt, 2 + r0v:2 + r0v + 8, 2:2 + W],
                            in_=ps[:].rearrange("p (r c) -> p r c", c=W),
                            func=ACTF.Identity, bias=b1s[mt][:, 0:1], scale=1.0)

                def emit_vpodd_rows(lo_row, hi_row):
                    # vpodd[p, v, i] = vpad[p, v, i+1]; segment rows chosen so
                    # consecutive segments do not overlap (no WAR with the
                    # previous quarter's odd-tap reads)
                    lo = lo_row * PW
                    hi = min(hi_row * PW, PH * PW) - 1
                    vflat = vpad[:].rearrange("p v a b -> p v (a b)")
                    nc.scalar.activation(out=vpodd[:, :, lo:hi],
                                         in_=vflat[:, :, lo + 1:hi + 1],
                                         func=ACTF.Copy)

                def emit_vpodd_seg(qq):
                    emit_vpodd_rows(qq * 16 + (4 if qq else 0), qq * 16 + 20)

                vblock_sched = {0: [0, 1, 2], 1: [3, 4], 2: [5, 6], 3: [7]}

                def build_chunk_units(chk, pool_prods=((1, 1), (2, 2)),
                                      tents_on_pool=False):
                    """Closure units for A-map chunk chk (4 pixel-tiles)."""
                    st = {}
                    units = []

                    def u_om(pi):
                        if pi == 0:
                            st["om_t"] = ombuf.tile([128, cpt, OM], bf16,
                                                    name="om_t")
                        pt = chk * cpt + pi
                        ps = psB.tile([128, OM], f32, name="omm")
                        for kt in range(2):
                            nc.tensor.matmul(
                                ps, lhsT=xs[kt][:, pt * 128:(pt + 1) * 128],
                                rhs=wtoms[kt][:, :],
                                start=(kt == 0), stop=False)
                        nc.tensor.matmul(
                            ps, lhsT=ones[0:1, :],
                            rhs=bom1[0:1, :], start=False, stop=True)
                        nc.scalar.activation(out=st["om_t"][:, pi, :], in_=ps,
                                             func=ACTF.Copy)
                    for pi in range(cpt):
                        units.append(lambda pi=pi: u_om(pi))

                    def u_tents():
                        om_t = st["om_t"]
                        oh = om_t[:, :, 0:144]
                        ow = om_t[:, :, 144:288]
                        st["mbf"] = om_t[:, :, 288:432]
                        th = [tbuf.tile([128, cpt, 144], bf16, name=f"th_{i}")
                              for i in range(3)]
                        tw = [tbuf.tile([128, cpt, 144], bf16, name=f"tw_{i}")
                              for i in range(3)]
                        st["th"], st["tw"] = th, tw
                        # tents (bf16): index 0,1,2 <-> i=-1,0,+1
                        # t(-1)=relu(-o); t(+1)=relu(o); slot1 holds NEGATED
                        # t(0): |o|-1 = relu(o)+relu(-o)-1.
                        if tents_on_pool:
                            # startup chunks: Act is the critical path there,
                            # Pool has slack
                            nc.gpsimd.tensor_scalar_max(out=th[2], in0=oh,
                                                        scalar1=0.0)
                            nc.gpsimd.tensor_scalar(
                                out=th[0], in0=oh, scalar1=-1.0, scalar2=0.0,
                                op0=ALU.mult, op1=ALU.max)
                            nc.gpsimd.tensor_scalar_max(out=tw[2], in0=ow,
                                                        scalar1=0.0)
                            nc.gpsimd.tensor_scalar(
                                out=tw[0], in0=ow, scalar1=-1.0, scalar2=0.0,
                                op0=ALU.mult, op1=ALU.max)
                        else:
                            nc.scalar.activation(out=th[2], in_=oh,
                                                 func=ACTF.Relu)
                            nc.scalar.activation(out=th[0], in_=oh,
                                                 func=ACTF.Relu, scale=-1.0)
                            nc.scalar.activation(out=tw[2], in_=ow,
                                                 func=ACTF.Relu)
                            nc.scalar.activation(out=tw[0], in_=ow,
                                                 func=ACTF.Relu, scale=-1.0)
                        nc.vector.scalar_tensor_tensor(
                            out=th[1], in0=th[2], scalar=-1.0, in1=th[0],
                            op0=ALU.add, op1=ALU.add)
                        nc.vector.scalar_tensor_tensor(
                            out=tw[1], in0=tw[2], scalar=-1.0, in1=tw[0],
                            op0=ALU.add, op1=ALU.add)
                    units.append(u_tents)

                    def u_masks():
                        for i in range(3):
                            nc.vector.tensor_tensor(out=st["th"][i],
                                                    in0=st["th"][i],
                                                    in1=st["mbf"], op=ALU.mult)
                    units.append(u_masks)

                    def u_memset():
                        # (0,0) scatter is a fresh write covering taps r<3,s<3;
                        # zero only the complement
                        at = atbuf.tile([128, cpt, TPAD * 16], bf16, name="at")
                        st["at"] = at
                        a_ap = at[:, :, :]
                        nc.gpsimd.memset(bass.AP(
                            a_ap.tensor, a_ap.offset + 3 * 16,
                            [[cpt * TPAD * 16, 128], [TPAD * 16, cpt],
                             [5 * 16, 3], [1, 32]]), 0.0)
                        nc.gpsimd.memset(bass.AP(
                            a_ap.tensor, a_ap.offset + 15 * 16,
                            [[cpt * TPAD * 16, 128], [TPAD * 16, cpt],
                             [1, 160]]), 0.0)
                    units.append(u_memset)

                    def u_prod(i, j):
                        peng = nc.gpsimd if (i, j) in pool_prods else nc.vector
                        prod = prodab.tile([128, cpt, 144], bf16, name="prodb")
                        peng.tensor_tensor(out=prod, in0=st["th"][i],
                                           in1=st["tw"][j], op=ALU.mult)
                        a_ap = st["at"][:, :, :]
                        o_ap = bass.AP(
                            a_ap.tensor, a_ap.offset + (i * 5 + j) * 16,
                            [[cpt * TPAD * 16, 128], [TPAD * 16, cpt],
                             [5 * 16, 3], [1, 48]])
                        p_ap = prod[:, :, :]
                        i_ap = bass.AP(
                            p_ap.tensor, p_ap.offset,
                            [[cpt * 144, 128], [144, cpt], [48, 3], [1, 48]])
                        if (i, j) == (0, 0):
                            nc.vector.tensor_copy(out=o_ap, in_=i_ap)
                        else:
                            sop = ALU.subtract if (i == 1) != (j == 1) else ALU.add
                            nc.vector.tensor_tensor(out=o_ap, in0=o_ap,
                                                    in1=i_ap, op=sop)
                    for i in range(3):
                        for j in range(3):
                            units.append(lambda i=i, j=j: u_prod(i, j))

                    def u_transpose(tb):
                        tps = trps.tile([128, 512], bf16, name="tr")
                        for s in range(4):
                            nc.tensor.transpose(
                                tps[:, s * 128:(s + 1) * 128],
                                st["at"][:, s, tb * 128:(tb + 1) * 128],
                                idn[:, :])
                        col = chk * cpt * 128
                        nc.scalar.activation(
                            out=atile[tb][:, col:col + 512], in_=tps,
                            func=ACTF.Copy)
                    for tb in range(4):
                        units.append(lambda tb=tb: u_transpose(tb))
                    return units

                def u_abc(qq, st, t, h=None):
                    # broadcast DMA for quarter qq, tap t (may be emitted a
                    # quarter early as a prefetch)
                    npix = QPIX if h is None else 512
                    coff = 0 if h is None else h * 512
                    tb, ts = t // 8, t % 8
                    abc = abcp.tile([128, npix], bf16, name="abc")
                    st["abc", t, h] = abc
                    a_ap2 = atile[tb][:, :]
                    sap = bass.AP(
                        a_ap2.tensor,
                        a_ap2.offset + ts * 16 * HW + qq * QPIX + coff,
                        [[HW, 16], [0, 8], [1, npix]])
                    nc.sync.dma_start(out=abc, in_=sap)

                def tap_half_units(qq, upsp, st, h=None):
                    """Tap units for quarter qq; h=None -> full 1024-pixel
                    quarter, h=0/1 -> 512-pixel half (own abc DMAs and
                    per-column-region PSUM chains)."""
                    pool_taps = pool_taps_q[qq]
                    units = []
                    r0 = qq * 16
                    order = list(range(len(taps)))

                    def u_tap(t, seq, h):
                        eh, ew = taps[t]
                        if seq == 0:
                            if h is None:
                                st["ups"] = [upsp.tile([128, QPIX], f32,
                                                       name=f"ups_{v}")
                                             for v in range(2)]
                            else:
                                st["ups", h] = [upsp.tile([128, 512], f32,
                                                          name=f"ups{h}_{v}")
                                                for v in range(2)]
                        ups = st["ups"] if h is None else st["ups", h]
                        npix = QPIX if h is None else 512
                        rows = npix // W
                        if ("abc", t, h) not in st:
                            u_abc(qq, st, t, h)
                        abc = st.pop(("abc", t, h))
                        # A operand repeated over both channel tiles via a
                        # stride-0 dim
                        b_ap = abc[:, :]
                        abc4 = bass.AP(
                            b_ap.tensor, b_ap.offset,
                            [[npix, 128], [0, 2], [W, rows], [1, W]])
                        rbase = 2 + r0 + (0 if h is None else h * 8) + eh
                        if ew % 2 == 0:
                            win = vpad[:, :, rbase:rbase + rows,
                                       2 + ew:2 + ew + W]
                        else:
                            vp4 = vpodd[:].rearrange("p v (a b) -> p v a b",
                                                     b=PW)
                            win = vp4[:, :, rbase:rbase + rows,
                                      1 + ew:1 + ew + W]
                        eng = nc.gpsimd if t in pool_taps else nc.vector
                        pr = prodp.tile([128, 2, rows, W], bf16, name="tp")
                        eng.tensor_tensor(out=pr, in0=win, in1=abc4,
                                          op=ALU.mult)
                        prf = pr[:].rearrange("p v a b -> p (v a b)")
                        for vt in range(2):
                            for nb in range(npix // 512):
                                nc.tensor.matmul(
                                    ups[vt][:, nb * 512:(nb + 1) * 512],
                                    lhsT=idn[:, :],
                                    rhs=prf[:, vt * npix + nb * 512:
                                            vt * npix + (nb + 1) * 512],
                                    start=(seq == 0),
                                    stop=(seq == len(taps) - 1))
                    for seq, t in enumerate(order):
                        units.append(lambda t=t, seq=seq, h=h: u_tap(t, seq, h))
                    return units

                def u_usb(st, usb_out, h=None):
                    if h is None or h == 0:
                        st["usb"] = usbp.tile([128, 2, QPIX], bf16, name="usb")
                        usb_out.append(st["usb"])
                    lo = 0 if h is None else h * 512
                    hi = QPIX if h is None else (h + 1) * 512
                    ups = st["ups"] if h is None else st["ups", h]
                    for vt in range(2):
                        nc.scalar.activation(out=st["usb"][:, vt, lo:hi],
                                             in_=ups[vt][:, 0:hi - lo],
                                             func=ACTF.Copy)

                def cv2_units(qq, usb, cvps, ysbp, nlocs=(0, 1)):
                    units = []
                    ysbs = {}

                    def u_cv2(nloc, mt):
                        if (nloc, mt % 2) not in ysbs and nloc == 0:
                            pass
                        ps2 = cvps.tile([128, 512], f32, name="cv2ps")
                        for kt in range(2):
                            nc.tensor.matmul(
                                ps2,
                                lhsT=wt2s[kt][:, mt * 128:(mt + 1) * 128],
                                rhs=usb[:, kt, nloc * 512:(nloc + 1) * 512],
                                start=(kt == 0), stop=(kt == 1))
                        if mt not in ysbs:
                            ysbs[mt] = ysbp.tile([128, QPIX], bf16,
                                                 name=f"ysb{mt}")
                        ysb = ysbs[mt]
                        nc.scalar.activation(out=ysb[:, nloc * 512:
                                                     (nloc + 1) * 512],
                                             in_=ps2, func=ACTF.Silu,
                                             bias=b2s[mt][:, 0:1], scale=1.0)
                        if nloc == 1:
                            nc.scalar.dma_start(
                                out=y_d[mt * 128:(mt + 1) * 128,
                                        qq * QPIX:(qq + 1) * QPIX],
                                in_=ysb)
                    for nloc in nlocs:
                        for mt in range(2):
                            units.append(lambda n=nloc, m=mt: u_cv2(n, m))
                    return units

                def emit_interleaved(primary, secondary, pace=1.0):
                    # pace < 1 front-loads: secondary exhausted after that
                    # fraction of primary
                    si, n_s = 0, len(secondary)
                    n_p = max(1, int(len(primary) * pace))
                    for k, p in enumerate(primary):
                        p()
                        want = min(n_s, ((k + 1) * n_s) // n_p)
                        while si < want:
                            secondary[si]()
                            si += 1
                    while si < n_s:
                        secondary[si]()
                        si += 1

                with tc.tile_pool(name="abcp", bufs=8) as abcp, \
                     tc.tile_pool(name="prodp", bufs=8) as prodp, \
                     tc.tile_pool(name="usbp", bufs=2) as usbp, \
                     tc.tile_pool(name="ysbp", bufs=2) as ysbp:

                    if phase >= 2:
                        # startup: A chunks 0,1 pipelined pairwise + V rows
                        # for quarter 0
                        for u in build_chunk_units(0, startup_pool_prods):
                            u()
                        su = build_chunk_units(1, startup_pool_prods)
                        vb = [lambda: emit_vblock(0), lambda: emit_vblock(1),
                              lambda: emit_vblock(2),
                              lambda: emit_vpodd_rows(0, 20)]
                        emit_interleaved(su, vb)

                    usbs = {}
                    sts = {q: {} for q in range(4)}
                    for qq in range(4 if phase >= 3 else 0):
                        # next-quarter build units
                        nxt = []
                        if qq < 3:
                            nxt += build_chunk_units(2 * qq + 2)
                            nxt += build_chunk_units(2 * qq + 3)
                            for ntv in vblock_sched[qq + 1]:
                                nxt.append(lambda ntv=ntv: emit_vblock(ntv))
                            nxt.append(lambda qq=qq: emit_vpodd_seg(qq + 1))
                            for tpre in range(n_prefetch):
                                nxt.append(lambda q2=qq + 1, t=tpre:
                                           u_abc(q2, sts[q2], t))

                        # previous quarter's cv2 in its own PSUM window
                        n_pre = 0
                        if qq > 0 and phase >= 4:
                            with tc.tile_pool(name=f"cvps{qq - 1}", bufs=2,
                                              space="PSUM") as cvps:
                                n_pre = min(n_pre, len(nxt))
                                emit_interleaved(
                                    cv2_units(qq - 1, usbs[qq - 1], cvps, ysbp),
                                    nxt[:n_pre])

                        if qq == 3:
                            # chunk 7's transposes (emitted in quarter 2's
                            # stream) are the last trps use; free its banks
                            trps_stack.close()

                        with tc.tile_pool(name=f"ups{qq}", bufs=1,
                                          space="PSUM") as upsp:
                            st = sts[qq]
                            usb_out = []
                            prim = (tap_half_units(qq, upsp, st)
                                    + [lambda: u_usb(st, usb_out)])
                            emit_interleaved(prim, nxt[n_pre:], pace=pace)
                            usbs[qq] = usb_out[0]

                    if phase >= 4:
                        with tc.tile_pool(name="cvps3", bufs=2,
                                          space="PSUM") as cvps:
                            for u in cv2_units(3, usbs[3], cvps, ysbp):
                                u()

    _split_multiwait(nc, mybir)
    return nc


def _prepare(inputs):
    x = np.ascontiguousarray(np.asarray(inputs["x"], np.float32))
    w_cv1 = np.asarray(inputs["w_cv1"], np.float32)
    b_cv1 = np.asarray(inputs["b_cv1"], np.float32)
    w_off = np.asarray(inputs["w_off"], np.float32)
    b_off = np.asarray(inputs["b_off"], np.float32)
    w_cv2 = np.asarray(inputs["w_cv2"], np.float32)
    bn_g = np.asarray(inputs["bn_gamma"], np.float32)
    bn_b = np.asarray(inputs["bn_beta"], np.float32)
    bn_m = np.asarray(inputs["bn_mean"], np.float32)
    bn_v = np.asarray(inputs["bn_var"], np.float32)

    perm_v = _v_perm()
    W1p = w_cv1[perm_v, :]
    b1p = b_cv1[perm_v]

    Wom = w_off @ w_cv1
    bom = w_off @ b_cv1 + b_off
    omp = _om_perm()
    Wom_p = Wom[omp]
    bom_p = bom[omp]

    s = bn_g / np.sqrt(bn_v + BN_EPS)
    W2s = w_cv2 * s[:, None]
    b2f = bn_b - bn_m * s
    W2p = W2s[:, perm_v]

    shared = dict(
        wt1=np.ascontiguousarray(W1p.T),
        wtom=np.ascontiguousarray(Wom_p.T),
        wt2=np.ascontiguousarray(W2p.T).astype(ml_dtypes.bfloat16),
        b1=np.ascontiguousarray(b1p[:, None]),
        b2=np.ascontiguousarray(b2f[:, None]),
        bom=np.ascontiguousarray(bom_p[None, :]),
        idn=np.eye(128, dtype=ml_dtypes.bfloat16),
        onesrow=np.ones((1, 128), np.float32),
    )
    in_maps = []
    for b in range(B):
        m = dict(shared)
        m["x"] = np.ascontiguousarray(x[b].reshape(C1, HW))
        in_maps.append(m)
    return in_maps


def kernel(**inputs):
    from concourse.bass_utils import run_bass_kernel_spmd

    if "nc" not in _cache:
        _cache["nc"] = _build_nc()
    nc = _cache["nc"]
    in_maps = _prepare(inputs)
    res = run_bass_kernel_spmd(nc, in_maps, core_ids=list(range(B)))
    out = np.stack([np.asarray(r["y"], np.float32).reshape(C2, H, W)
                    for r in res.results])
    return out


if __name__ == "__main__":
    rng = np.random.default_rng(0)
    demo = dict(
        x=rng.standard_normal((B, C1, H, W)).astype(np.float32),
        w_cv1=rng.standard_normal((C, C1)).astype(np.float32) / 16,
        b_cv1=(rng.standard_normal((C,)) * 0.1).astype(np.float32),
        w_off=(rng.standard_normal((G * 3 * K, C)) * 0.01).astype(np.float32),
        b_off=(rng.standard_normal((G * 3 * K,)) * 0.01).astype(np.float32),
        w_cv2=rng.standard_normal((C2, C)).astype(np.float32) / 16,
        bn_gamma=rng.uniform(0.5, 1.5, (C2,)).astype(np.float32),
        bn_beta=(rng.standard_normal((C2,)) * 0.1).astype(np.float32),
        bn_mean=(rng.standard_normal((C2,)) * 0.1).astype(np.float32),
        bn_var=rng.uniform(0.5, 1.5, (C2,)).astype(np.float32),
    )
    y = kernel(**demo)
    print("kernel ran, output", y.shape, y.dtype)


# revision 30
# speedup vs baseline: 1.1272x; 1.0118x over previous
"""DCNv4 block (cv1 1x1 -> offset/mask proj -> deformable bilinear sampling
-> cv2 1x1 -> BN -> SiLU) as a Bass/Tile kernel for Trainium2.

Strategy
--------
Data-parallel over batch: each of the 8 NeuronCores processes one image.

The deformable sampling is reformulated gather-free: with |off| < 1 the
bilinear sample of kernel point k at (h+kh+off_h, w+kw+off_w) equals
  sum_{i,j in {-1,0,1}} tent(off_h - i) * tent(off_w - j) * V[h+kh+i, w+kw+j]
with tent(t) = max(0, 1-|t|).  Merging all 9 kernel points over absolute
displacements e=(eh,ew) in [-2,2]^2 gives 25 "taps":
  out[p,g,:] = sum_e A_e[p,g] * Vpad[p+e, g, :]
  A_e[p,g]   = sum_k mask_k * tent(off_h - (eh-kh)) * tent(off_w - (ew-kw))
Out-of-image corners are handled exactly by zero-padding Vpad (the reference
drops those corners).

Engine mapping:
 - PE: cv1 / offset-projection / cv2 matmuls (float32r), A^T transposes,
   and the 25-term tap accumulation as identity-weight matmuls accumulating
   into PSUM (f32 accumulation).
 - DVE: tent products, A scatter-build, most per-tap elementwise A*V
   products (both 128-channel tiles fused into one [128,2048] op via a
   stride-0 repeat on the A operand).
 - ACT: tents read the offset projection directly from PSUM; cv1 bias is
   applied by the PSUM->SBUF copy's per-partition bias; BN+SiLU epilogue.
 - GPSIMD: 7 of 25 tap products per quarter, 2 A-build products, memsets
   (trimmed to only the A slots not freshly written by the (0,0) scatter,
   which is a copy instead of an accumulate).
 - DMA: a replicating access pattern broadcasts per-group tap maps A_e[g,:]
   (16 partitions) to all 128 partitions (V channels are laid out g-major,
   partition j -> group j//8, so one broadcast serves both channel tiles).

BN is folded into cv2 on the host; the offset projection is folded through
cv1 on the host so offsets are computed from x at full precision independent
of V's bf16 rounding.  The output DMA is bf16 (cast to f32 on host).
"""

import sys
import numpy as np

if "/opt/trn_rl_repo" not in sys.path:
    sys.path.insert(0, "/opt/trn_rl_repo")

import ml_dtypes

B, C1, C2, H, W = 8, 256, 256, 64, 64
C = 256
G = 16
Cg = 16
K = 9
HW = H * W           # 4096
PW = W + 4           # 68
PH = H + 4
BN_EPS = 1e-5
TPAD = 32            # taps padded to 32 so (t, g) blocks are 128-aligned
OM = 432             # offset/mask projection width (3*K*G)

_cache = {}


def _v_perm():
    # vtile vt, partition j  ->  original channel g*16 + c
    perm = []
    for vt in range(2):
        for j in range(128):
            g = j // 8
            c = vt * 8 + (j % 8)
            perm.append(g * Cg + c)
    return np.array(perm, np.int64)


def _om_perm():
    # om channel r (0..431) -> original w_off row
    rows = np.zeros(432, np.int64)
    for r in range(144):
        k, g = r // 16, r % 16
        rows[r] = g * 27 + 2 * k            # off_h (dh)
        rows[144 + r] = g * 27 + 2 * k + 1  # off_w (dw)
        rows[288 + r] = g * 27 + 18 + k     # mask
    return rows


def _split_multiwait(nc, mybir, max_waits=1):
    """walrus in this container rejects >1 sem wait on one instruction;
    split extras onto preceding same-engine NoOps (equivalent ordering)."""
    for f in nc.m.functions:
        for bb in f.blocks:
            out = []
            for inst in bb.instructions:
                si = inst.sync_info
                if si is not None and len(si.on_wait) > max_waits:
                    waits = list(si.on_wait)
                    for w in waits[:-max_waits]:
                        nop = mybir.InstNoOp(
                            name=f"I-nopw{nc.next_id()}", ins=[], outs=[])
                        nop.engine = inst.engine
                        nop.sync_info = mybir.SyncInfo(on_wait=[w], on_update=[])
                        nc.register_instruction(nop)
                        out.append(nop)
                    si.on_wait = waits[-max_waits:]
                out.append(inst)
            bb.instructions = out


def _build_nc(phase=99, pace=0.85, n_pre=6, n_prefetch=0, pool_taps_q=((1, 5, 9, 13, 16, 19, 23),) * 3 + ((1, 5, 9, 16, 19, 23),),
              startup_pool_prods=((1, 1), (2, 2), (0, 1))):
    import concourse.bass as bass
    import concourse.mybir as mybir
    import concourse.tile as tile

    f32 = mybir.dt.float32
    f32r = mybir.dt.float32r
    bf16 = mybir.dt.bfloat16
    ALU = mybir.AluOpType
    ACTF = mybir.ActivationFunctionType

    nc = bass.Bass()

    x_d = nc.dram_tensor("x", [C1, HW], f32r, kind="ExternalInput")
    wt1_d = nc.dram_tensor("wt1", [C1, 256], f32r, kind="ExternalInput")
    wtom_d = nc.dram_tensor("wtom", [C1, OM], f32r, kind="ExternalInput")
    wt2_d = nc.dram_tensor("wt2", [C, C2], bf16, kind="ExternalInput")
    b1_d = nc.dram_tensor("b1", [C, 1], f32, kind="ExternalInput")
    b2_d = nc.dram_tensor("b2", [C2, 1], f32, kind="ExternalInput")
    bom_d = nc.dram_tensor("bom", [1, OM], f32r, kind="ExternalInput")
    idn_d = nc.dram_tensor("idn", [128, 128], bf16, kind="ExternalInput")
    ones_d = nc.dram_tensor("onesrow", [1, 128], f32r, kind="ExternalInput")
    y_d = nc.dram_tensor("y", [C2, HW], bf16, kind="ExternalOutput")

    with tile.TileContext(nc) as tc:
        with tc.tile_pool(name="persist", bufs=1) as persist:

            # ---- persistent tiles ----
            wt1s = [persist.tile([128, 256], f32r, name=f"wt1_{i}") for i in range(2)]
            wtoms = [persist.tile([128, OM], f32r, name=f"wtom_{i}") for i in range(2)]
            wt2s = [persist.tile([128, 256], bf16, name=f"wt2_{i}") for i in range(2)]
            b1s = [persist.tile([128, 1], f32, name=f"b1_{i}") for i in range(2)]
            bom1 = persist.tile([1, OM], f32r, name="bom1")
            b2s = [persist.tile([128, 1], f32, name=f"b2_{i}") for i in range(2)]
            ones = persist.tile([1, 128], f32r, name="ones")
            idn = persist.tile([128, 128], bf16, name="idn")
            # V and its 1-col-shifted copy, both channel tiles fused:
            # [128 part, vt, padded rows, padded cols]
            vpad = persist.tile([128, 2, PH, PW], bf16, name="vpad")
            vpodd = persist.tile([128, 2, PH * PW], bf16, name="vpodd")
            atile = [persist.tile([128, HW], bf16, name=f"atile_{i}") for i in range(4)]

            for i in range(2):
                nc.sync.dma_start(out=wtoms[i], in_=wtom_d[i * 128:(i + 1) * 128, :])
            nc.sync.dma_start(out=bom1, in_=bom_d[:, :])
            nc.sync.dma_start(out=ones, in_=ones_d[:, :])

            # zero the pad ring of Vpad (interior written by cv1)
            nc.vector.memset(vpad[:, :, 0:2, :], 0.0)
            nc.vector.memset(vpad[:, :, PH - 2:PH, :], 0.0)
            nc.vector.memset(vpad[:, :, 2:PH - 2, 0:2], 0.0)
            nc.vector.memset(vpad[:, :, 2:PH - 2, PW - 4:PW], 0.0)

            with tc.tile_pool(name="build", bufs=1) as bpool, \
                 tc.tile_pool(name="tbuf", bufs=2) as tbuf, \
                 tc.tile_pool(name="ombuf", bufs=2) as ombuf, \
                 tc.tile_pool(name="prodab", bufs=4) as prodab, \
                 tc.tile_pool(name="atbuf", bufs=2) as atbuf, \
                 tc.tile_pool(name="psB", bufs=2, space="PSUM") as psB:

                from contextlib import ExitStack
                trps_stack = ExitStack()
                trps = trps_stack.enter_context(
                    tc.tile_pool(name="trps", bufs=2, space="PSUM"))

                xs = [bpool.tile([128, HW], f32r, name=f"xs_{i}") for i in range(2)]

                for i in range(2):
                    nc.sync.dma_start(
                        out=xs[i][:, 0:1024],
                        in_=x_d[i * 128:(i + 1) * 128, 0:1024])
                nc.sync.dma_start(out=idn, in_=idn_d[:, :])
                for i in range(2):
                    nc.sync.dma_start(out=wt1s[i], in_=wt1_d[i * 128:(i + 1) * 128, :])
                    nc.sync.dma_start(out=b1s[i], in_=b1_d[i * 128:(i + 1) * 128, :])
                for i in range(2):
                    nc.sync.dma_start(
                        out=xs[i][:, 1024:HW],
                        in_=x_d[i * 128:(i + 1) * 128, 1024:HW])
                for i in range(2):
                    nc.sync.dma_start(out=wt2s[i], in_=wt2_d[i * 128:(i + 1) * 128, :])
                    nc.sync.dma_start(out=b2s[i], in_=b2_d[i * 128:(i + 1) * 128, :])

                taps = [(eh, ew) for eh in range(-2, 3) for ew in range(-2, 3)]
                QPIX = 1024          # pixels per tap quarter (16 image rows)
                cpt = 4

                def emit_vblock(nt):
                    # cv1 rows nt*8..nt*8+8 for both channel tiles; bias is
                    # applied by the PSUM->SBUF activation copy
                    for mt in range(2):
                        ps = psB.tile([128, 512], f32, name="omm")
                        for kt in range(2):
                            nc.tensor.matmul(
                                ps, lhsT=wt1s[kt][:, mt * 128:(mt + 1) * 128],
                                rhs=xs[kt][:, nt * 512:(nt + 1) * 512],
                                start=(kt == 0), stop=(kt == 1))
                        r0v = nt * 8
                        nc.scalar.activation(
                            out=vpad[:, mt, 2 + r0v:2 + r0v + 8, 2:2 + W],
                            in_=ps[:].rearrange("p (r c) -> p r c", c=W),
                            func=ACTF.Identity, bias=b1s[mt][:, 0:1], scale=1.0)

                def emit_vpodd_rows(lo_row, hi_row):
                    # vpodd[p, v, i] = vpad[p, v, i+1]; segment rows chosen so
                    # consecutive segments do not overlap (no WAR with the
                    # previous quarter's odd-tap reads)
                    lo = lo_row * PW
                    hi = min(hi_row * PW, PH * PW) - 1
                    vflat = vpad[:].rearrange("p v a b -> p v (a b)")
                    nc.scalar.activation(out=vpodd[:, :, lo:hi],
                                         in_=vflat[:, :, lo + 1:hi + 1],
                                         func=ACTF.Copy)

                def emit_vpodd_seg(qq):
                    emit_vpodd_rows(qq * 16 + (4 if qq else 0), qq * 16 + 20)

                vblock_sched = {0: [0, 1, 2], 1: [3, 4], 2: [5, 6], 3: [7]}

                def build_chunk_units(chk, pool_prods=((1, 1), (2, 2)),
                                      tents_on_pool=False):
                    """Closure units for A-map chunk chk (4 pixel-tiles)."""
                    st = {}
                    units = []

                    def u_om(pi):
                        if pi == 0:
                            st["om_t"] = ombuf.tile([128, cpt, OM], bf16,
                                                    name="om_t")
                        pt = chk * cpt + pi
                        ps = psB.tile([128, OM], f32, name="omm")
                        for kt in range(2):
                            nc.tensor.matmul(
                                ps, lhsT=xs[kt][:, pt * 128:(pt + 1) * 128],
                                rhs=wtoms[kt][:, :],
                                start=(kt == 0), stop=False)
                        nc.tensor.matmul(
                            ps, lhsT=ones[0:1, :],
                            rhs=bom1[0:1, :], start=False, stop=True)
                        nc.scalar.activation(out=st["om_t"][:, pi, :], in_=ps,
                                             func=ACTF.Copy)
                    for pi in range(cpt):
                        units.append(lambda pi=pi: u_om(pi))

                    def u_tents():
                        om_t = st["om_t"]
                        oh = om_t[:, :, 0:144]
                        ow = om_t[:, :, 144:288]
                        st["mbf"] = om_t[:, :, 288:432]
                        th = [tbuf.tile([128, cpt, 144], bf16, name=f"th_{i}")
                              for i in range(3)]
                        tw = [tbuf.tile([128, cpt, 144], bf16, name=f"tw_{i}")
                              for i in range(3)]
                        st["th"], st["tw"] = th, tw
                        # tents (bf16): index 0,1,2 <-> i=-1,0,+1
                        # t(-1)=relu(-o); t(+1)=relu(o); slot1 holds NEGATED
                        # t(0): |o|-1 = relu(o)+relu(-o)-1.
                        if tents_on_pool:
                            # startup chunks: Act is the critical path there,
                            # Pool has slack
                            nc.gpsimd.tensor_scalar_max(out=th[2], in0=oh,
                                                        scalar1=0.0)
                            nc.gpsimd.tensor_scalar(
                                out=th[0], in0=oh, scalar1=-1.0, scalar2=0.0,
                                op0=ALU.mult, op1=ALU.max)
                            nc.gpsimd.tensor_scalar_max(out=tw[2], in0=ow,
                                                        scalar1=0.0)
                            nc.gpsimd.tensor_scalar(
                                out=tw[0], in0=ow, scalar1=-1.0, scalar2=0.0,
                                op0=ALU.mult, op1=ALU.max)
                        else:
                            nc.scalar.activation(out=th[2], in_=oh,
                                                 func=ACTF.Relu)
                            nc.scalar.activation(out=th[0], in_=oh,
                                                 func=ACTF.Relu, scale=-1.0)
                            nc.scalar.activation(out=tw[2], in_=ow,
                                                 func=ACTF.Relu)
                            nc.scalar.activation(out=tw[0], in_=ow,
                                                 func=ACTF.Relu, scale=-1.0)
                        nc.vector.scalar_tensor_tensor(
                            out=th[1], in0=th[2], scalar=-1.0, in1=th[0],
                            op0=ALU.add, op1=ALU.add)
                        nc.vector.scalar_tensor_tensor(
                            out=tw[1], in0=tw[2], scalar=-1.0, in1=tw[0],
                            op0=ALU.add, op1=ALU.add)
                    units.append(u_tents)

                    def u_masks():
                        for i in range(3):
                            nc.vector.tensor_tensor(out=st["th"][i],
                                                    in0=st["th"][i],
                                                    in1=st["mbf"], op=ALU.mult)
                    units.append(u_masks)

                    def u_memset():
                        # (0,0) scatter is a fresh write covering taps r<3,s<3;
                        # zero only the complement
                        at = atbuf.tile([128, cpt, TPAD * 16], bf16, name="at")
                        st["at"] = at
                        a_ap = at[:, :, :]
                        nc.gpsimd.memset(bass.AP(
                            a_ap.tensor, a_ap.offset + 3 * 16,
                            [[cpt * TPAD * 16, 128], [TPAD * 16, cpt],
                             [5 * 16, 3], [1, 32]]), 0.0)
                        nc.gpsimd.memset(bass.AP(
                            a_ap.tensor, a_ap.offset + 15 * 16,
                            [[cpt * TPAD * 16, 128], [TPAD * 16, cpt],
                             [1, 160]]), 0.0)
                    units.append(u_memset)

                    def u_prod(i, j):
                        peng = nc.gpsimd if (i, j) in pool_prods else nc.vector
                        prod = prodab.tile([128, cpt, 144], bf16, name="prodb")
                        peng.tensor_tensor(out=prod, in0=st["th"][i],
                                           in1=st["tw"][j], op=ALU.mult)
                        a_ap = st["at"][:, :, :]
                        o_ap = bass.AP(
                            a_ap.tensor, a_ap.offset + (i * 5 + j) * 16,
                            [[cpt * TPAD * 16, 128], [TPAD * 16, cpt],
                             [5 * 16, 3], [1, 48]])
                        p_ap = prod[:, :, :]
                        i_ap = bass.AP(
                            p_ap.tensor, p_ap.offset,
                            [[cpt * 144, 128], [144, cpt], [48, 3], [1, 48]])
                        if (i, j) == (0, 0):
                            nc.vector.tensor_copy(out=o_ap, in_=i_ap)
                        else:
                            sop = ALU.subtract if (i == 1) != (j == 1) else ALU.add
                            nc.vector.tensor_tensor(out=o_ap, in0=o_ap,
                                                    in1=i_ap, op=sop)
                    for i in range(3):
                        for j in range(3):
                            units.append(lambda i=i, j=j: u_prod(i, j))

                    def u_transpose(tb):
                        tps = trps.tile([128, 512], bf16, name="tr")
                        for s in range(4):
                            nc.tensor.transpose(
                                tps[:, s * 128:(s + 1) * 128],
                                st["at"][:, s, tb * 128:(tb + 1) * 128],
                                idn[:, :])
                        col = chk * cpt * 128
                        nc.scalar.activation(
                            out=atile[tb][:, col:col + 512], in_=tps,
                            func=ACTF.Copy)
                    for tb in range(4):
                        units.append(lambda tb=tb: u_transpose(tb))
                    return units

                def u_abc(qq, st, t, h=None):
                    # broadcast DMA for quarter qq, tap t (may be emitted a
                    # quarter early as a prefetch)
                    npix = QPIX if h is None else 512
                    coff = 0 if h is None else h * 512
                    tb, ts = t // 8, t % 8
                    abc = abcp.tile([128, npix], bf16, name="abc")
                    st["abc", t, h] = abc
                    a_ap2 = atile[tb][:, :]
                    sap = bass.AP(
                        a_ap2.tensor,
                        a_ap2.offset + ts * 16 * HW + qq * QPIX + coff,
                        [[HW, 16], [0, 8], [1, npix]])
                    nc.sync.dma_start(out=abc, in_=sap)

                def tap_half_units(qq, upsp, st, h=None):
                    """Tap units for quarter qq; h=None -> full 1024-pixel
                    quarter, h=0/1 -> 512-pixel half (own abc DMAs and
                    per-column-region PSUM chains)."""
                    pool_taps = pool_taps_q[qq]
                    units = []
                    r0 = qq * 16
                    order = list(range(len(taps)))

                    def u_tap(t, seq, h):
                        eh, ew = taps[t]
                        if seq == 0:
                            if h is None:
                                st["ups"] = [upsp.tile([128, QPIX], f32,
                                                       name=f"ups_{v}")
                                             for v in range(2)]
                            else:
                                st["ups", h] = [upsp.tile([128, 512], f32,
                                                          name=f"ups{h}_{v}")
                                                for v in range(2)]
                        ups = st["ups"] if h is None else st["ups", h]
                        npix = QPIX if h is None else 512
                        rows = npix // W
                        if ("abc", t, h) not in st:
                            u_abc(qq, st, t, h)
                        abc = st.pop(("abc", t, h))
                        # A operand repeated over both channel tiles via a
                        # stride-0 dim
                        b_ap = abc[:, :]
                        abc4 = bass.AP(
                            b_ap.tensor, b_ap.offset,
                            [[npix, 128], [0, 2], [W, rows], [1, W]])
                        rbase = 2 + r0 + (0 if h is None else h * 8) + eh
                        if ew % 2 == 0:
                            win = vpad[:, :, rbase:rbase + rows,
                                       2 + ew:2 + ew + W]
                        else:
                            vp4 = vpodd[:].rearrange("p v (a b) -> p v a b",
                                                     b=PW)
                            win = vp4[:, :, rbase:rbase + rows,
                                      1 + ew:1 + ew + W]
                        eng = nc.gpsimd if t in pool_taps else nc.vector
                        pr = prodp.tile([128, 2, rows, W], bf16, name="tp")
                        eng.tensor_tensor(out=pr, in0=win, in1=abc4,
                                          op=ALU.mult)
                        prf = pr[:].rearrange("p v a b -> p (v a b)")
                        for vt in range(2):
                            for nb in range(npix // 512):
                                nc.tensor.matmul(
                                    ups[vt][:, nb * 512:(nb + 1) * 512],
                                    lhsT=idn[:, :],
                                    rhs=prf[:, vt * npix + nb * 512:
                                            vt * npix + (nb + 1) * 512],
                                    start=(seq == 0),
                                    stop=(seq == len(taps) - 1))
                    for seq, t in enumerate(order):
                        units.append(lambda t=t, seq=seq, h=h: u_tap(t, seq, h))
                    return units

                def u_usb(st, usb_out, h=None):
                    if h is None or h == 0:
                        st["usb"] = usbp.tile([128, 2, QPIX], bf16, name="usb")
                        usb_out.append(st["usb"])
                    lo = 0 if h is None else h * 512
                    hi = QPIX if h is None else (h + 1) * 512
                    ups = st["ups"] if h is None else st["ups", h]
                    for vt in range(2):
                        nc.scalar.activation(out=st["usb"][:, vt, lo:hi],
                                             in_=ups[vt][:, 0:hi - lo],
                                             func=ACTF.Copy)

                def cv2_units(qq, usb, cvps, ysbp, nlocs=(0, 1)):
                    units = []
                    ysbs = {}

                    def u_cv2(nloc, mt):
                        if (nloc, mt % 2) not in ysbs and nloc == 0:
                            pass
                        ps2 = cvps.tile([128, 512], f32, name="cv2ps")
                        for kt in range(2):
                            nc.tensor.matmul(
                                ps2,
                                lhsT=wt2s[kt][:, mt * 128:(mt + 1) * 128],
                                rhs=usb[:, kt, nloc * 512:(nloc + 1) * 512],
                                start=(kt == 0), stop=(kt == 1))
                        if mt not in ysbs:
                            ysbs[mt] = ysbp.tile([128, QPIX], bf16,
                                                 name=f"ysb{mt}")
                        ysb = ysbs[mt]
                        nc.scalar.activation(out=ysb[:, nloc * 512:
                                                     (nloc + 1) * 512],
                                             in_=ps2, func=ACTF.Silu,
                                             bias=b2s[mt][:, 0:1], scale=1.0)
                        if nloc == 1:
                            nc.scalar.dma_start(
                                out=y_d[mt * 128:(mt + 1) * 128,
                                        qq * QPIX:(qq + 1) * QPIX],
                                in_=ysb)
                    for nloc in nlocs:
                        for mt in range(2):
                            units.append(lambda n=nloc, m=mt: u_cv2(n, m))
                    return units

                def emit_interleaved(primary, secondary, pace=1.0):
                    # pace < 1 front-loads: secondary exhausted after that
                    # fraction of primary
                    si, n_s = 0, len(secondary)
                    n_p = max(1, int(len(primary) * pace))
                    for k, p in enumerate(primary):
                        p()
                        want = min(n_s, ((k + 1) * n_s) // n_p)
                        while si < want:
                            secondary[si]()
                            si += 1
                    while si < n_s:
                        secondary[si]()
                        si += 1

                with tc.tile_pool(name="abcp", bufs=8) as abcp, \
                     tc.tile_pool(name="prodp", bufs=8) as prodp, \
                     tc.tile_pool(name="usbp", bufs=2) as usbp, \
                     tc.tile_pool(name="ysbp", bufs=2) as ysbp:

                    if phase >= 2:
                        # startup: A chunks 0,1 pipelined pairwise + V rows
                        # for quarter 0
                        for u in build_chunk_units(0, startup_pool_prods):
                            u()
                        su = build_chunk_units(1, startup_pool_prods)
                        vb = [lambda: emit_vblock(0), lambda: emit_vblock(1),
                              lambda: emit_vblock(2),
                              lambda: emit_vpodd_rows(0, 20)]
                        emit_interleaved(su, vb)

                    usbs = {}
                    sts = {q: {} for q in range(4)}
                    for qq in range(4 if phase >= 3 else 0):
                        # next-quarter build units
                        nxt = []
                        if qq < 3:
                            nxt += build_chunk_units(2 * qq + 2)
                            nxt += build_chunk_units(2 * qq + 3)
                            for ntv in vblock_sched[qq + 1]:
                                nxt.append(lambda ntv=ntv: emit_vblock(ntv))
                            nxt.append(lambda qq=qq: emit_vpodd_seg(qq + 1))
                            for tpre in range(n_prefetch):
                                nxt.append(lambda q2=qq + 1, t=tpre:
                                           u_abc(q2, sts[q2], t))

                        # previous quarter's cv2 in its own PSUM window
                        n_pre = 0
                        if qq > 0 and phase >= 4:
                            with tc.tile_pool(name=f"cvps{qq - 1}", bufs=2,
                                              space="PSUM") as cvps:
                                n_pre = min(n_pre, len(nxt))
                                emit_interleaved(
                                    cv2_units(qq - 1, usbs[qq - 1], cvps, ysbp),
                                    nxt[:n_pre])

                        if qq == 3:
                            # chunk 7's transposes (emitted in quarter 2's
                            # stream) are the last trps use; free its banks
                            trps_stack.close()

                        with tc.tile_pool(name=f"ups{qq}", bufs=1,
                                          space="PSUM") as upsp:
                            st = sts[qq]
                            usb_out = []
                            prim = (tap_half_units(qq, upsp, st)
                                    + [lambda: u_usb(st, usb_out)])
                            emit_interleaved(prim, nxt[n_pre:], pace=pace)
                            usbs[qq] = usb_out[0]

                    if phase >= 4:
                        with tc.tile_pool(name="cvps3", bufs=2,
                                          space="PSUM") as cvps:
                            for u in cv2_units(3, usbs[3], cvps, ysbp):
                                u()

    _split_multiwait(nc, mybir)
    return nc


def _prepare(inputs):
    x = np.ascontiguousarray(np.asarray(inputs["x"], np.float32))
    w_cv1 = np.asarray(inputs["w_cv1"], np.float32)
    b_cv1 = np.asarray(inputs["b_cv1"], np.float32)
    w_off = np.asarray(inputs["w_off"], np.float32)
    b_off = np.asarray(inputs["b_off"], np.float32)
    w_cv2 = np.asarray(inputs["w_cv2"], np.float32)
    bn_g = np.asarray(inputs["bn_gamma"], np.float32)
    bn_b = np.asarray(inputs["bn_beta"], np.float32)
    bn_m = np.asarray(inputs["bn_mean"], np.float32)
    bn_v = np.asarray(inputs["bn_var"], np.float32)

    perm_v = _v_perm()
    W1p = w_cv1[perm_v, :]
    b1p = b_cv1[perm_v]

    Wom = w_off @ w_cv1
    bom = w_off @ b_cv1 + b_off
    omp = _om_perm()
    Wom_p = Wom[omp]
    bom_p = bom[omp]

    s = bn_g / np.sqrt(bn_v + BN_EPS)
    W2s = w_cv2 * s[:, None]
    b2f = bn_b - bn_m * s
    W2p = W2s[:, perm_v]

    shared = dict(
        wt1=np.ascontiguousarray(W1p.T),
        wtom=np.ascontiguousarray(Wom_p.T),
        wt2=np.ascontiguousarray(W2p.T).astype(ml_dtypes.bfloat16),
        b1=np.ascontiguousarray(b1p[:, None]),
        b2=np.ascontiguousarray(b2f[:, None]),
        bom=np.ascontiguousarray(bom_p[None, :]),
        idn=np.eye(128, dtype=ml_dtypes.bfloat16),
        onesrow=np.ones((1, 128), np.float32),
    )
    in_maps = []
    for b in range(B):
        m = dict(shared)
        m["x"] = np.ascontiguousarray(x[b].reshape(C1, HW))
        in_maps.append(m)
    return in_maps


def kernel(**inputs):
    from concourse.bass_utils import run_bass_kernel_spmd

    if "nc" not in _cache:
        _cache["nc"] = _build_nc()
    nc = _cache["nc"]
    in_maps = _prepare(inputs)
    res = run_bass_kernel_spmd(nc, in_maps, core_ids=list(range(B)))
    out = np.stack([np.asarray(r["y"], np.float32).reshape(C2, H, W)
                    for r in res.results])
    return out


if __name__ == "__main__":
    rng = np.random.default_rng(0)
    demo = dict(
        x=rng.standard_normal((B, C1, H, W)).astype(np.float32),
        w_cv1=rng.standard_normal((C, C1)).astype(np.float32) / 16,
        b_cv1=(rng.standard_normal((C,)) * 0.1).astype(np.float32),
        w_off=(rng.standard_normal((G * 3 * K, C)) * 0.01).astype(np.float32),
        b_off=(rng.standard_normal((G * 3 * K,)) * 0.01).astype(np.float32),
        w_cv2=rng.standard_normal((C2, C)).astype(np.float32) / 16,
        bn_gamma=rng.uniform(0.5, 1.5, (C2,)).astype(np.float32),
        bn_beta=(rng.standard_normal((C2,)) * 0.1).astype(np.float32),
        bn_mean=(rng.standard_normal((C2,)) * 0.1).astype(np.float32),
        bn_var=rng.uniform(0.5, 1.5, (C2,)).astype(np.float32),
    )
    y = kernel(**demo)
    print("kernel ran, output", y.shape, y.dtype)
